# revision 46
# baseline (speedup 1.0000x reference)
"""GCN (3-layer, symmetric-normalized) on 8 Trainium2 NeuronCores.

Strategy
--------
z_l = A_n @ (h_l W_l) + b_l with A_n = D^-1/2 (A+I) D^-1/2.  We factor the
edge norm into the node tables:  table_l = dinv * (h_l W_l)  (rows scaled by
dinv[src]), aggregate with a 0/1 selection matmul per 128-dst window
(S^T @ msg accumulated in PSUM), and apply dinv[dst] afterwards.  The bias is
injected into PSUM as a rank-1 matmul sqrt(deg) x b so that the final scale
dinv * (Z + sqrtdeg x b) = dinv*Z + b.

Sharding: dst nodes are permuted (degree-balanced bins) into 8 x 98 windows of
128; each core owns 98 windows and all edges targeting them.  Layer-1 table is
computed redundantly on every core (cheaper than an AllGather of 100MB);
layer-2/3 tables are computed shard-wise and AllGathered (25MB).

Gathers use dma_gather (int16 indices, 16-partition-wrapped, replicated x8);
the table is split into 4 row-groups of 32768 so indices fit int16.  Matmuls
run in float32r (TF32-like, ~1.6e-4 rel err, 4x faster than fp32).
"""

import math

import numpy as np

try:
    import concourse  # noqa: F401
except ImportError:  # pragma: no cover
    import sys

    sys.path.insert(0, "/opt/trn_rl_repo")

import concourse.bass as bass
import concourse.bacc as bacc
import concourse.mybir as mybir
import concourse.tile as tile
from concourse import bass_utils
from concourse.masks import make_identity

F32 = mybir.dt.float32
F32R = mybir.dt.float32r
BF16 = mybir.dt.bfloat16
I16 = mybir.dt.int16

import os

N_CORES = 8
DEBUG_TAPS = False
PHASES = os.environ.get("K_PHASES", "BCDEFG")  # debug: which phases to build
# single_packet=True concatenates each DMA lane's descriptors into one packet;
# HW caps a packet at 64 descriptors, so it hangs when num_idxs/16 + 1 > 64.
# Keep num_idxs <= 7*128 = 896 (56 descs/lane) per gather call and it is safe.
SINGLE_PACKET = os.environ.get("K_SINGLE_PACKET", "1") == "1"
GATHER_MAX_CHUNKS = int(os.environ.get("K_GATHER_MAX_CHUNKS", "7"))
# bf16 for x/W1/layer-1 table: halves gather + input DMA traffic.
USE_BF16 = os.environ.get("K_BF16", "1") == "1"


class Cfg:
    def __init__(self, N, NB, F_IN, H1, H2, C, QSHIFT):
        assert NB % N_CORES == 0
        self.N = N                    # real nodes
        self.NB = NB                  # total 128-node windows (bins)
        self.NP = NB * 128            # padded nodes
        self.NWC = NB // N_CORES      # windows per core
        self.SHARD = self.NWC * 128   # rows per core
        self.F_IN = F_IN              # input features (mult of 128)
        self.H1 = H1                  # layer-1 width (mult of 128)
        self.H2 = H2                  # layer-2 width (<=128)
        self.C = C                    # classes (<=16)
        self.QSHIFT = QSHIFT          # group shift (rows per group = 1<<QSHIFT)
        self.QROWS = 1 << QSHIFT
        self.NG = (self.NP + self.QROWS - 1) >> QSHIFT
        assert F_IN % 128 == 0 and H1 % 128 == 0 and H2 <= 128 and C <= 16


FULL_CFG = Cfg(N=100000, NB=784, F_IN=512, H1=256, H2=64, C=10, QSHIFT=15)


# --------------------------------------------------------------------------
# host-side graph preprocessing
# --------------------------------------------------------------------------
def _preprocess(x, edge_index, cfg):
    N, NP, NB = cfg.N, cfg.NP, cfg.NB
    NWC, SHARD, NG = cfg.NWC, cfg.SHARD, cfg.NG

    src = np.asarray(edge_index[0], dtype=np.int64)
    dst = np.asarray(edge_index[1], dtype=np.int64)
    loops = np.arange(N, dtype=np.int64)
    src = np.concatenate([src, loops])
    dst = np.concatenate([dst, loops])
    E = src.shape[0]

    deg = np.bincount(dst, minlength=N).astype(np.int64)
    degp = np.concatenate([deg, np.ones(NP - N, dtype=np.int64)])

    # ---- degree-balanced node permutation: snake-deal into NB bins --------
    order = np.argsort(-degp, kind="stable")          # nodes by degree desc
    i = np.arange(NP)
    r = i // NB                                       # deal round = slot
    cpos = i % NB
    binid = np.where(r % 2 == 0, cpos, NB - 1 - cpos)
    load = np.bincount(binid, weights=degp[order].astype(np.float64), minlength=NB)
    border = np.argsort(-load, kind="stable")         # bins by load desc
    bin_core = np.empty(NB, dtype=np.int64)
    bin_w = np.empty(NB, dtype=np.int64)
    bin_core[border] = np.arange(NB) % N_CORES
    bin_w[border] = np.arange(NB) // N_CORES
    perm_row = np.empty(NP, dtype=np.int64)
    perm_row[order] = bin_core[binid] * SHARD + bin_w[binid] * 128 + r
    inv_perm = np.empty(NP, dtype=np.int64)
    inv_perm[perm_row] = np.arange(NP)

    # permuted per-node arrays
    x = np.asarray(x, dtype=np.float32)
    xp = np.zeros((NP, cfg.F_IN), dtype=np.float32)
    real = inv_perm < N
    xp[real] = x[inv_perm[real]]
    deg_perm = degp[inv_perm].astype(np.float32)      # [NP]

    # x transposed tiles: XT[t, k, kb, p] = xp[t*128+p, kb*128+k]
    KB = cfg.F_IN // 128
    XT = np.ascontiguousarray(
        xp.reshape(NB, 128, KB, 128).transpose(0, 3, 2, 1)
    )  # [NB, 128, KB, 128]
    # 4-tile interleave so one DMA loads 4 tiles contiguously per partition:
    # XT4[q, k, i, kb, p] = XT[4q+i, k, kb, p]
    XT = np.ascontiguousarray(
        XT.reshape(NB // 4, 4, 128, KB, 128).transpose(0, 2, 1, 3, 4)
    )  # [NB/4, 128, 4, KB, 128]
    if USE_BF16:
        XT = XT.astype(mybir.dt.np(BF16))

    # ---- edges -> (core, window, group), sorted ---------------------------
    es = perm_row[src]
    ed = perm_row[dst]
    core_e = ed // SHARD
    w_e = (ed % SHARD) // 128
    slot_e = ed % 128
    grp_e = es >> cfg.QSHIFT
    # sub-split each (w, g) by dst-slot half so every non-first chunk's
    # slots live in one 64-slot block (narrow selection matmuls at legal
    # PE tile positions); slot-sorted within each (core, w, g2)
    NG2 = NG * 2
    g2_e = grp_e * 2 + (slot_e >= 64)
    key = ((core_e * NWC + w_e) * NG2 + g2_e).astype(np.int64)
    eorder = np.argsort(key * 128 + slot_e, kind="stable")
    key_s = key[eorder]
    es_s = es[eorder]
    slot_i = slot_e[eorder]

    counts = np.bincount(key, minlength=N_CORES * NWC * NG2).reshape(
        N_CORES, NWC, NG2
    )
    # shared (all-core) padded sizes per (window, group-half)
    max_cg = counts.max(axis=0)                       # [NWC, NG2]
    n_pad = 128 * ((max_cg + 127) // 128)             # [NWC, NG2] multiple of 128
    cpw_g = n_pad // 128
    cpw_w = cpw_g.sum(axis=1)                         # chunks per window
    tot_cpw = int(cpw_w.sum())
    chunk_off_wg = np.zeros((NWC, NG2), dtype=np.int64)
    chunk_off_w = np.zeros(NWC, dtype=np.int64)
    acc = 0
    for w in range(NWC):
        chunk_off_w[w] = acc
        for g in range(NG2):
            chunk_off_wg[w, g] = acc
            acc += cpw_g[w, g]
    assert acc == tot_cpw

    # per-edge position within its (core, w, g2) run
    gstart = np.zeros(N_CORES * NWC * NG2 + 1, dtype=np.int64)
    np.cumsum(counts.reshape(-1), out=gstart[1:])
    pos = np.arange(E, dtype=np.int64) - gstart[key_s]

    # destination columns in the flat arrays (same offsets on every core)
    wg = key_s % (NWC * NG2)                          # (w, g2) combined
    w_s = wg // NG2
    g_s = wg % NG2
    chunk = chunk_off_wg[w_s, g_s] + pos // 128       # global chunk column
    part = pos % 128
    core_s = key_s // (NWC * NG2)

    TOT_COLS = tot_cpw * 8                            # int16 cols (128 idx -> 8)
    idx_flat = np.zeros((N_CORES, 16, TOT_COLS), dtype=np.int16)
    idx1_flat = np.zeros((N_CORES, 16, TOT_COLS), dtype=np.int16)
    dstloc_flat = np.full((N_CORES, 128, tot_cpw), -1.0, dtype=np.float32)

    # idx position within group = pos; wrapped [16, n/16] at group col offset
    icol = chunk_off_wg[w_s, g_s] * 8 + pos // 16
    ipart = pos % 16
    qbase = (es_s >> cfg.QSHIFT) << cfg.QSHIFT
    lval = es_s - qbase
    ival = lval.astype(np.int16)
    # layer-1 table G1 is stored 4-tile interleaved [NB/4, 128, 4, H1]:
    # physical row of local row l is (l//512)*512 + (l%128)*4 + (l//128)%4
    pval = ((lval >> 9) << 9) + ((lval & 127) << 2) + ((lval >> 7) & 3)
    idx_flat[core_s, ipart, icol] = ival
    idx1_flat[core_s, ipart, icol] = pval.astype(np.int16)

    # dstloc holds absolute slots (full-width selection matrices: the ISA
    # only allows matmul PSUM writes at partition offset 0)
    dstloc_flat[core_s, part, chunk] = slot_i.astype(np.float32)
    idx_flat = np.tile(idx_flat, (1, 8, 1))           # replicate to 128 partitions
    idx1_flat = np.tile(idx1_flat, (1, 8, 1))

    # per-core deg arrays
    deg_shard = np.empty((N_CORES, 128, NWC), dtype=np.float32)
    degT_row = np.empty((N_CORES, 1, SHARD), dtype=np.float32)
    deg_full = np.ascontiguousarray(
        deg_perm.reshape(NB, 128).T
    )  # [128, NB] col t = tile t
    for c in range(N_CORES):
        sh = deg_perm[c * SHARD : (c + 1) * SHARD]
        deg_shard[c] = sh.reshape(NWC, 128).T
        degT_row[c, 0] = sh

    iota = np.broadcast_to(np.arange(128, dtype=np.float32), (128, 128)).copy()

    meta = dict(
        cpw_g=cpw_g, cpw_w=cpw_w, chunk_off_wg=chunk_off_wg,
        chunk_off_w=chunk_off_w, tot_cpw=tot_cpw, tot_cols=TOT_COLS,
    )
    host = dict(
        XT=XT, deg_full=deg_full, iota=iota,
        idx_flat=idx_flat, idx1_flat=idx1_flat, dstloc_flat=dstloc_flat,
        deg_shard=deg_shard, degT_row=degT_row,
        inv_perm=inv_perm, perm_row=perm_row,
    )
    return host, meta


# --------------------------------------------------------------------------
# device program
# --------------------------------------------------------------------------
def _build_program(cfg, meta):
    NB, NWC, SHARD = cfg.NB, cfg.NWC, cfg.SHARD
    F_IN, H1, H2, C = cfg.F_IN, cfg.H1, cfg.H2, cfg.C
    KB = F_IN // 128
    KB2 = H1 // 128
    NG = cfg.NG
    cpw_g = meta["cpw_g"]
    cpw_w = meta["cpw_w"]
    chunk_off_wg = meta["chunk_off_wg"]
    chunk_off_w = meta["chunk_off_w"]
    TOT_CPW = meta["tot_cpw"]
    TOT_COLS = meta["tot_cols"]
    MAXCPW = int(cpw_w.max())

    nc = bacc.Bacc("TRN2", target_bir_lowering=False, debug=False,
                   num_devices=N_CORES)

    WDT = BF16 if USE_BF16 else F32
    # ---- I/O ---------------------------------------------------------------
    XT = nc.dram_tensor("XT", [NB // 4, 128, 4, KB, 128], WDT,
                        kind="ExternalInput")
    W1 = nc.dram_tensor("W1", [128, KB, H1], WDT, kind="ExternalInput")
    W2 = nc.dram_tensor("W2", [128, KB2, H2], F32, kind="ExternalInput")
    W3 = nc.dram_tensor("W3", [H2, 16], F32, kind="ExternalInput")
    B1 = nc.dram_tensor("B1", [1, H1], F32, kind="ExternalInput")
    B2 = nc.dram_tensor("B2", [1, H2], F32, kind="ExternalInput")
    B3 = nc.dram_tensor("B3", [1, 16], F32, kind="ExternalInput")
    IOTA = nc.dram_tensor("IOTA", [128, 128], F32, kind="ExternalInput")
    DEGF = nc.dram_tensor("DEGF", [128, NB], F32, kind="ExternalInput")
    DEGS = nc.dram_tensor("DEGS", [128, NWC], F32, kind="ExternalInput")
    IDX = nc.dram_tensor("IDX", [128, TOT_COLS], I16, kind="ExternalInput")
    IDX1 = nc.dram_tensor("IDX1", [128, TOT_COLS], I16, kind="ExternalInput")
    DSTL = nc.dram_tensor("DSTL", [128, TOT_CPW], F32, kind="ExternalInput")
    OUT = nc.dram_tensor("OUT", [SHARD, C], F32, kind="ExternalOutput")

    # ---- internal DRAM -----------------------------------------------------
    G1 = nc.dram_tensor("G1", [NB // 4, 128, 4, H1], WDT)
    G2S = nc.dram_tensor("G2S", [SHARD, H2], F32)
    G2F = nc.dram_tensor("G2F", [cfg.NP, H2], F32, addr_space="Shared")
    G3S = nc.dram_tensor("G3S", [SHARD, H2], F32)
    G3F = nc.dram_tensor("G3F", [cfg.NP, H2], F32, addr_space="Shared")

    rg = [list(range(N_CORES))]

    with tile.TileContext(nc) as tc:
        # ---------- resident constants ----------
        with tc.tile_pool(name="const", bufs=1) as cp:
            identf = cp.tile([128, 128], F32)
            make_identity(nc, identf[:])
            ident = cp.tile([128, 128], F32R)
            nc.vector.tensor_copy(out=ident[:], in_=identf[:])
            iota = cp.tile([128, 128], F32)
            nc.sync.dma_start(out=iota[:], in_=IOTA[:])
            RT = BF16 if USE_BF16 else F32R
            w1 = cp.tile([128, KB, H1], RT)
            nc.sync.dma_start(
                out=w1[:], in_=W1[:] if USE_BF16 else W1[:].bitcast(F32R))
            w2 = cp.tile([128, KB2, H2], F32R)
            nc.sync.dma_start(out=w2[:], in_=W2[:].bitcast(F32R))
            w3 = cp.tile([H2, 16], F32R)
            nc.sync.dma_start(out=w3[:], in_=W3[:].bitcast(F32R))
            b1 = cp.tile([1, H1], F32R)
            nc.sync.dma_start(out=b1[:], in_=B1[:].bitcast(F32R))
            b2 = cp.tile([1, H2], F32R)
            nc.sync.dma_start(out=b2[:], in_=B2[:].bitcast(F32R))
            b3 = cp.tile([1, 16], F32R)
            nc.sync.dma_start(out=b3[:], in_=B3[:].bitcast(F32R))

            degf = cp.tile([128, NB], F32)
            nc.sync.dma_start(out=degf[:], in_=DEGF[:])
            sqf = cp.tile([128, NB], F32)
            nc.scalar.sqrt(out=sqf[:], in_=degf[:])
            dinvf = cp.tile([128, NB], F32)
            nc.vector.reciprocal(out=dinvf[:], in_=sqf[:])

            degs = cp.tile([128, NWC], F32)
            nc.sync.dma_start(out=degs[:], in_=DEGS[:])
            sqs = cp.tile([128, NWC], F32)
            nc.scalar.sqrt(out=sqs[:], in_=degs[:])
            dinvs = cp.tile([128, NWC], F32)
            nc.vector.reciprocal(out=dinvs[:], in_=sqs[:])
            deginvs = cp.tile([128, NWC], F32)
            nc.vector.reciprocal(out=deginvs[:], in_=degs[:])
            # sqrt(deg) column form; per-window rows made via PE transpose
            sq_colr = cp.tile([128, NWC], F32R)
            nc.vector.tensor_copy(out=sq_colr[:], in_=sqs[:])

            dstl = cp.tile([128, TOT_CPW], F32)
            nc.sync.dma_start(out=dstl[:], in_=DSTL[:])
            # flat-layout edge indices resident for phases E and G
            idxfull = cp.tile([128, TOT_COLS], I16)
            nc.sync.dma_start(out=idxfull[:], in_=IDX[:])

            # phase-G softmax staging (batched ln => one act-table switch)
            TTA = cp.tile([128, NWC, C], F32)
            SSA = cp.tile([128, NWC], F32)
            LSA = cp.tile([128, NWC], F32)
            OOA = cp.tile([128, NWC, C], F32)

            # ---------- phase B: table1 = dinv * (x @ W1), all rows ----------
            # 4 tiles per DMA in and out (XT and G1 are 4-tile interleaved)
            with tc.tile_pool(name="l1", bufs=3) as l1p, \
                 tc.tile_pool(name="l1ps", bufs=4, space="PSUM") as l1ps:
                for t4 in range(NB // 4 if "B" in PHASES else 0):
                    if USE_BF16:
                        xtr = l1p.tile([128, 4, KB, 128], BF16, tag="xt")
                        nc.sync.dma_start(out=xtr[:], in_=XT[t4])
                    else:
                        xt = l1p.tile([128, 4, KB, 128], F32, tag="xt")
                        nc.sync.dma_start(out=xt[:], in_=XT[t4])
                        xtr = l1p.tile([128, 4, KB, 128], F32R, tag="xtr")
                        nc.vector.tensor_copy(out=xtr[:], in_=xt[:])
                    g1q = l1p.tile([128, 4, H1], RT, tag="g1q")
                    for i in range(4):
                        t = t4 * 4 + i
                        ps = l1ps.tile([128, H1], F32, space="PSUM", tag="ps")
                        for kb in range(KB):
                            nc.tensor.matmul(out=ps[:], lhsT=xtr[:, i, kb, :],
                                             rhs=w1[:, kb, :],
                                             start=(kb == 0),
                                             stop=(kb == KB - 1))
                        nc.scalar.activation(
                            out=g1q[:, i, :], in_=ps[:],
                            func=mybir.ActivationFunctionType.Copy,
                            scale=dinvf[:, t : t + 1])
                    g1dst = G1[t4]
                    nc.sync.dma_start(
                        out=g1dst if USE_BF16 else g1dst.bitcast(F32R),
                        in_=g1q[:])

            # ---------- per-layer aggregation ----------
            def aggregate(w, tview, fdim, agp, agps, tag, stop_last=False,
                          mdt=F32R, idx_dram=None):
                """Accumulate S^T @ msg for window w into a PSUM tile [128, fdim]."""
                cpw = int(cpw_w[w])
                coff = int(chunk_off_w[w])
                if idx_dram is not None:
                    idxt = agp.tile([128, MAXCPW * 8], I16, tag=tag + "idx")
                    nc.sync.dma_start(
                        out=idxt[:, : cpw * 8],
                        in_=idx_dram[:, coff * 8 : (coff + cpw) * 8])

                    def iap(a, b):
                        return idxt[:, (a - coff) * 8 : (b - coff) * 8]
                else:
                    def iap(a, b):
                        return idxfull[:, a * 8 : b * 8]
                msg = agp.tile([128, MAXCPW, fdim], mdt, tag=tag + "msg")
                for g in range(NG * 2):
                    cg = int(cpw_g[w, g])
                    if cg == 0:
                        continue
                    goff = int(chunk_off_wg[w, g]) - coff
                    qlo = (g >> 1) << cfg.QSHIFT
                    qhi = min(qlo + cfg.QROWS, cfg.NP)
                    step = GATHER_MAX_CHUNKS if GATHER_MAX_CHUNKS else cg
                    tbl = tview(qlo, qhi)
                    if tbl.dtype != mdt:
                        tbl = tbl.bitcast(mdt)
                    gc0 = int(chunk_off_wg[w, g])
                    for c0 in range(0, cg, step):
                        cn = min(step, cg - c0)
                        nc.gpsimd.dma_gather(
                            out_ap=msg[:, goff + c0 : goff + c0 + cn, :],
                            in_ap=tbl,
                            idxs_ap=iap(gc0 + c0, gc0 + c0 + cn),
                            num_idxs=cn * 128,
                            num_idxs_reg=cn * 128,
                            elem_size=fdim,
                            single_packet=SINGLE_PACKET,
                        )
                sdt = mdt if mdt is BF16 else F32R
                S = agp.tile([128, MAXCPW, 128], sdt, tag=tag + "S")
                nc.vector.tensor_tensor(
                    out=S[:, :cpw, :],
                    in0=dstl[:, coff : coff + cpw].unsqueeze(2)
                        .to_broadcast([128, cpw, 128]),
                    in1=iota[:].unsqueeze(1).to_broadcast([128, cpw, 128]),
                    op=mybir.AluOpType.is_equal)
                Z = agps.tile([128, fdim], F32, space="PSUM", tag=tag + "Z")
                for j in range(cpw):
                    nc.tensor.matmul(out=Z[:], lhsT=S[:, j, :],
                                     rhs=msg[:, j, :], start=(j == 0),
                                     stop=(stop_last and j == cpw - 1))
                return Z

            def sqrtdeg_row(w, agp, sqps, tag):
                """sqrt(deg) of window w as a [1, 128] f32r row (PE transpose)."""
                pt = sqps.tile([1, 128], F32R, space="PSUM", tag=tag + "sqT")
                nc.tensor.transpose(out=pt[:], in_=sq_colr[:, w : w + 1],
                                    identity=ident[:])
                row = agp.tile([1, 128], F32R, tag=tag + "sqr")
                nc.vector.tensor_copy(out=row[:], in_=pt[:])
                return row

            # ---------- phase C: layer-1 aggregation -> table2 shard ----------
            with tc.tile_pool(name="ag1", bufs=2) as agp, \
                 tc.tile_pool(name="ag1z", bufs=2, space="PSUM") as agps, \
                 tc.tile_pool(name="ag1q", bufs=1, space="PSUM") as sqps, \
                 tc.tile_pool(name="ag1t", bufs=2, space="PSUM") as trps:
                def g1view(qlo, qhi):
                    return G1[qlo // 512 : qhi // 512].flatten_outer_dims()

                for w in range(NWC if "C" in PHASES else 0):
                    Z = aggregate(w, g1view, H1, agp, agps, "c",
                                  mdt=BF16 if USE_BF16 else F32R,
                                  idx_dram=IDX1)
                    sqrow = sqrtdeg_row(w, agp, sqps, "c")
                    nc.tensor.matmul(out=Z[:], lhsT=sqrow[:], rhs=b1[:],
                                     start=False, stop=True)
                    h2 = agp.tile([128, H1], F32R, tag="ch2")
                    nc.scalar.activation(out=h2[:], in_=Z[:],
                                         func=mybir.ActivationFunctionType.Relu,
                                         scale=dinvs[:, w : w + 1])
                    h2T = agp.tile([128, KB2, 128], F32R, tag="ch2T")
                    for kb in range(KB2):
                        tp = trps.tile([128, 128], F32R, space="PSUM", tag="ctp")
                        nc.tensor.transpose(
                            out=tp[:], in_=h2[:, kb * 128 : (kb + 1) * 128],
                            identity=ident[:])
                        nc.vector.tensor_copy(out=h2T[:, kb, :], in_=tp[:])
                    g2p = agps.tile([128, H2], F32, space="PSUM", tag="cg2p")
                    for kb in range(KB2):
                        nc.tensor.matmul(out=g2p[:], lhsT=h2T[:, kb, :],
                                         rhs=w2[:, kb, :],
                                         start=(kb == 0), stop=(kb == KB2 - 1))
                    g2sb = agp.tile([128, H2], F32R, tag="cg2sb")
                    nc.vector.tensor_scalar(
                        out=g2sb[:], in0=g2p[:],
                        scalar1=dinvs[:, w : w + 1], scalar2=None,
                        op0=mybir.AluOpType.mult)
                    nc.sync.dma_start(
                        out=G2S[w * 128 : (w + 1) * 128, :].bitcast(F32R),
                        in_=g2sb[:])

            # ---------- phase D: AllGather table2 ----------
            if "D" in PHASES:
                nc.gpsimd.collective_compute(
                    "AllGather", mybir.AluOpType.bypass, replica_groups=rg,
                    ins=[G2S[:].opt()], outs=[G2F[:].opt()])

            # ---------- phase E: layer-2 aggregation -> table3 shard ----------
            with tc.tile_pool(name="ag2", bufs=2) as agp, \
                 tc.tile_pool(name="ag2z", bufs=2, space="PSUM") as agps, \
                 tc.tile_pool(name="ag2q", bufs=1, space="PSUM") as sqps:
                for w in range(NWC if "E" in PHASES else 0):
                    Z = aggregate(w, lambda a, b: G2F[a:b, :], H2, agp, agps,
                                  "e")
                    sqrow = sqrtdeg_row(w, agp, sqps, "e")
                    nc.tensor.matmul(out=Z[:], lhsT=sqrow[:], rhs=b2[:],
                                     start=False, stop=True)
                    h3 = agp.tile([128, H2], F32, tag="eh3")
                    nc.scalar.activation(out=h3[:], in_=Z[:],
                                         func=mybir.ActivationFunctionType.Relu)
                    g3sb = agp.tile([128, H2], F32R, tag="eg3sb")
                    nc.vector.tensor_scalar(
                        out=g3sb[:], in0=h3[:],
                        scalar1=deginvs[:, w : w + 1], scalar2=None,
                        op0=mybir.AluOpType.mult)
                    nc.sync.dma_start(
                        out=G3S[w * 128 : (w + 1) * 128, :].bitcast(F32R),
                        in_=g3sb[:])

            # ---------- phase F: AllGather table3 ----------
            if "F" in PHASES:
                nc.gpsimd.collective_compute(
                    "AllGather", mybir.AluOpType.bypass, replica_groups=rg,
                    ins=[G3S[:].opt()], outs=[G3F[:].opt()])

            # ---------- phase G: layer-3 aggregation + W3 + log_softmax ------
            with tc.tile_pool(name="ag3", bufs=2) as agp, \
                 tc.tile_pool(name="ag3z", bufs=2, space="PSUM") as agps, \
                 tc.tile_pool(name="ag3q", bufs=1, space="PSUM") as sqps, \
                 tc.tile_pool(name="ag3t", bufs=2, space="PSUM") as trps:
                for w in range(NWC if "G" in PHASES else 0):
                    Z = aggregate(w, lambda a, b: G3F[a:b, :], H2, agp, agps,
                                  "g", stop_last=True)
                    # close the accumulation group (dummy 0-add via stop on last)
                    z3 = agp.tile([128, H2], F32R, tag="gz3")
                    nc.scalar.activation(out=z3[:], in_=Z[:],
                                         func=mybir.ActivationFunctionType.Copy)
                    tp = trps.tile([H2, 128], F32R, space="PSUM", tag="gtp")
                    nc.tensor.transpose(out=tp[:], in_=z3[:], identity=ident[:])
                    z3T = agp.tile([H2, 128], F32R, tag="gz3T")
                    nc.vector.tensor_copy(out=z3T[:], in_=tp[:])
                    p3 = agps.tile([128, 16], F32, space="PSUM", tag="gp3")
                    nc.tensor.matmul(out=p3[:], lhsT=z3T[:], rhs=w3[:],
                                     start=True, stop=False)
                    sqrow = sqrtdeg_row(w, agp, sqps, "g")
                    nc.tensor.matmul(out=p3[:], lhsT=sqrow[:], rhs=b3[:],
                                     start=False, stop=True)
                    zf = agp.tile([128, 16], F32, tag="gzf")
                    nc.vector.tensor_scalar(
                        out=zf[:], in0=p3[:],
                        scalar1=dinvs[:, w : w + 1], scalar2=None,
                        op0=mybir.AluOpType.mult)
                    m = agp.tile([128, 1], F32, tag="gm")
                    nc.vector.reduce_max(out=m[:], in_=zf[:, :C],
                                         axis=mybir.AxisListType.X)
                    nc.vector.tensor_scalar(
                        out=TTA[:, w, :], in0=zf[:, :C], scalar1=m[:],
                        scalar2=None, op0=mybir.AluOpType.subtract)
                    ee = agp.tile([128, C], F32, tag="gee")
                    nc.scalar.activation(out=ee[:], in_=TTA[:, w, :],
                                         func=mybir.ActivationFunctionType.Exp,
                                         accum_out=SSA[:, w : w + 1])
                # single ln pass over all windows' exp-sums (one table load),
                # then subtract and store
                if "G" in PHASES:
                    nc.scalar.activation(out=LSA[:], in_=SSA[:],
                                         func=mybir.ActivationFunctionType.Ln)
                    nc.vector.tensor_tensor(
                        out=OOA[:],
                        in0=TTA[:],
                        in1=LSA[:].unsqueeze(2).to_broadcast([128, NWC, C]),
                        op=mybir.AluOpType.subtract)
                    for w in range(NWC):
                        nc.sync.dma_start(out=OUT[w * 128 : (w + 1) * 128, :],
                                          in_=OOA[:, w, :])

        if DEBUG_TAPS:
            DG1 = nc.dram_tensor("DG1", [cfg.NP, H1], F32, kind="ExternalOutput")
            DG2 = nc.dram_tensor("DG2", [SHARD, H2], F32, kind="ExternalOutput")
            DG3 = nc.dram_tensor("DG3", [SHARD, H2], F32, kind="ExternalOutput")
            nc.sync.dma_start(out=DG1[:], in_=G1[:])
            nc.sync.dma_start(out=DG2[:], in_=G2S[:])
            nc.sync.dma_start(out=DG3[:], in_=G3S[:])

    nc.compile()
    return nc


# --------------------------------------------------------------------------
# driver
# --------------------------------------------------------------------------
def _weights_layout(W1, W2, W3, b1, b2, b3, cfg):
    KB = cfg.F_IN // 128
    KB2 = cfg.H1 // 128
    W1t = np.ascontiguousarray(
        np.asarray(W1, np.float32).reshape(KB, 128, cfg.H1).transpose(1, 0, 2))
    W2t = np.ascontiguousarray(
        np.asarray(W2, np.float32).reshape(KB2, 128, cfg.H2).transpose(1, 0, 2))
    W3p = np.zeros((cfg.H2, 16), np.float32)
    W3p[:, : cfg.C] = np.asarray(W3, np.float32)
    B1 = np.asarray(b1, np.float32).reshape(1, cfg.H1)
    B2 = np.asarray(b2, np.float32).reshape(1, cfg.H2)
    B3 = np.zeros((1, 16), np.float32)
    B3[0, : cfg.C] = np.asarray(b3, np.float32)
    if USE_BF16:
        W1t = W1t.astype(mybir.dt.np(BF16))
    return W1t, W2t, W3p, B1, B2, B3


def prepare(x, edge_index, W1, b1, W2, b2, W3, b3, cfg=FULL_CFG):
    host, meta = _preprocess(x, edge_index, cfg)
    W1t, W2t, W3p, B1, B2, B3 = _weights_layout(W1, W2, W3, b1, b2, b3, cfg)
    nc = _build_program(cfg, meta)
    in_maps = []
    for c in range(N_CORES):
        in_maps.append({
            "XT": host["XT"], "W1": W1t, "W2": W2t, "W3": W3p,
            "B1": B1, "B2": B2, "B3": B3,
            "IOTA": host["iota"], "DEGF": host["deg_full"],
            "DEGS": host["deg_shard"][c],
            "IDX": host["idx_flat"][c], "IDX1": host["idx1_flat"][c],
            "DSTL": host["dstloc_flat"][c],
        })
    return nc, in_maps, host


def run(x, edge_index, W1, b1, W2, b2, W3, b3, cfg=FULL_CFG, trace=False):
    nc, in_maps, host = prepare(x, edge_index, W1, b1, W2, b2, W3, b3, cfg)
    res = bass_utils.run_bass_kernel_spmd(
        nc, in_maps, core_ids=list(range(N_CORES)), trace=trace)

    outp = np.concatenate([res.results[c]["OUT"] for c in range(N_CORES)], axis=0)
    out = outp[host["perm_row"][: cfg.N]]
    return out.astype(np.float32), res


def make_runner(cfg, nc, in_maps):
    """Build a reusable jitted 8-core runner with inputs resident on device.

    Returns (fn, args) where fn(*args) -> list of per-core OUT arrays; call
    repeatedly for timing without host->device transfer of the big inputs.
    """
    import jax
    from jax.sharding import Mesh, PartitionSpec
    from jax.experimental.shard_map import shard_map
    from concourse import bass2jax as b2j

    b2j.install_neuronx_cc_hook()
    partition_name = nc.partition_id_tensor.name if nc.partition_id_tensor else None
    in_names, out_names, out_avals, zero_outs = [], [], [], []
    for alloc in nc.m.functions[0].allocations:
        if not isinstance(alloc, mybir.MemoryLocationSet):
            continue
        name = alloc.memorylocations[0].name
        if alloc.kind == "ExternalInput":
            if name != partition_name:
                in_names.append(name)
        elif alloc.kind == "ExternalOutput":
            shape = tuple(alloc.tensor_shape)
            dtype = mybir.dt.np(alloc.dtype)
            out_names.append(name)
            out_avals.append(jax.core.ShapedArray(shape, dtype))
            zero_outs.append(np.zeros(shape, dtype))
    n_params = len(in_names)
    all_in_names = list(in_names) + out_names
    if partition_name is not None:
        all_in_names.append(partition_name)

    def _body(*args):
        operands = list(args)
        if partition_name is not None:
            operands.append(b2j.partition_id_tensor())
        outs = b2j._bass_exec_p.bind(
            *operands, out_avals=tuple(out_avals), in_names=tuple(all_in_names),
            out_names=tuple(out_names), lowering_input_output_aliases=(),
            sim_require_finite=False, sim_require_nnan=False, nc=nc)
        return tuple(outs)

    devices = jax.devices()[:N_CORES]
    mesh = Mesh(np.asarray(devices), ("core",))
    in_specs = (PartitionSpec("core"),) * (n_params + len(out_names))
    out_specs = (PartitionSpec("core"),) * len(out_names)
    sharded = jax.jit(
        shard_map(_body, mesh=mesh, in_specs=in_specs, out_specs=out_specs,
                  check_rep=False),
        keep_unused=True)
    sh = jax.sharding.NamedSharding(mesh, PartitionSpec("core"))
    concat_in = [
        jax.device_put(
            np.concatenate([np.asarray(in_maps[c][k]) for c in range(N_CORES)], 0),
            sh)
        for k in in_names
    ]
    concat_zeros = [
        jax.device_put(np.zeros((N_CORES * z.shape[0], *z.shape[1:]), z.dtype), sh)
        for z in zero_outs
    ]
    args = concat_in + concat_zeros

    def fn(*a):
        outs = sharded(*a)
        return {name: outs[i] for i, name in enumerate(out_names)}

    return fn, args, out_names


def kernel(**inputs):
    out, _ = run(
        inputs["x"], inputs["edge_index"],
        inputs["W1"], inputs["b1"], inputs["W2"], inputs["b2"],
        inputs["W3"], inputs["b3"], cfg=FULL_CFG)
    return out



# revision 62
# speedup vs baseline: 1.0789x; 1.0789x over previous
"""GCN (3-layer, symmetric-normalized) on 8 Trainium2 NeuronCores.

Strategy
--------
z_l = A_n @ (h_l W_l) + b_l with A_n = D^-1/2 (A+I) D^-1/2.  We factor the
edge norm into the node tables:  table_l = dinv * (h_l W_l)  (rows scaled by
dinv[src]), aggregate with a 0/1 selection matmul per 128-dst window
(S^T @ msg accumulated in PSUM), and apply dinv[dst] afterwards.  The bias is
injected into PSUM as a rank-1 matmul sqrt(deg) x b so that the final scale
dinv * (Z + sqrtdeg x b) = dinv*Z + b.

Sharding: dst nodes are permuted (degree-balanced bins) into 8 x 98 windows of
128; each core owns 98 windows and all edges targeting them.  Layer-1 table is
computed redundantly on every core (cheaper than an AllGather of 100MB);
layer-2/3 tables are computed shard-wise and AllGathered (25MB).

Gathers use dma_gather (int16 indices, 16-partition-wrapped, replicated x8);
the table is split into 4 row-groups of 32768 so indices fit int16.  Matmuls
run in float32r (TF32-like, ~1.6e-4 rel err, 4x faster than fp32).
"""

import math

import numpy as np

try:
    import concourse  # noqa: F401
except ImportError:  # pragma: no cover
    import sys

    sys.path.insert(0, "/opt/trn_rl_repo")

import concourse.bass as bass
import concourse.bacc as bacc
import concourse.mybir as mybir
import concourse.tile as tile
from concourse import bass_utils
from concourse.masks import make_identity

F32 = mybir.dt.float32
F32R = mybir.dt.float32r
BF16 = mybir.dt.bfloat16
I16 = mybir.dt.int16

import os

N_CORES = 8
DEBUG_TAPS = False
PHASES = os.environ.get("K_PHASES", "BCDEFG")  # debug: which phases to build
# single_packet=True concatenates each DMA lane's descriptors into one packet;
# HW caps a packet at 64 descriptors, so it hangs when num_idxs/16 + 1 > 64.
# Keep num_idxs <= 7*128 = 896 (56 descs/lane) per gather call and it is safe.
SINGLE_PACKET = os.environ.get("K_SINGLE_PACKET", "1") == "1"
GATHER_MAX_CHUNKS = int(os.environ.get("K_GATHER_MAX_CHUNKS", "7"))
# bf16 for x/W1/layer-1 table: halves gather + input DMA traffic.
USE_BF16 = os.environ.get("K_BF16", "1") == "1"


class Cfg:
    def __init__(self, N, NB, F_IN, H1, H2, C, QSHIFT):
        assert NB % N_CORES == 0
        self.N = N                    # real nodes
        self.NB = NB                  # total 128-node windows (bins)
        self.NP = NB * 128            # padded nodes
        self.NWC = NB // N_CORES      # windows per core
        self.SHARD = self.NWC * 128   # rows per core
        self.F_IN = F_IN              # input features (mult of 128)
        self.H1 = H1                  # layer-1 width (mult of 128)
        self.H2 = H2                  # layer-2 width (<=128)
        self.C = C                    # classes (<=16)
        self.QSHIFT = QSHIFT          # group shift (rows per group = 1<<QSHIFT)
        self.QROWS = 1 << QSHIFT
        self.NG = (self.NP + self.QROWS - 1) >> QSHIFT
        assert F_IN % 128 == 0 and H1 % 128 == 0 and H2 <= 128 and C <= 16


FULL_CFG = Cfg(N=100000, NB=784, F_IN=512, H1=256, H2=64, C=10, QSHIFT=15)


# --------------------------------------------------------------------------
# host-side graph preprocessing
# --------------------------------------------------------------------------
def _preprocess(x, edge_index, cfg):
    N, NP, NB = cfg.N, cfg.NP, cfg.NB
    NWC, SHARD, NG = cfg.NWC, cfg.SHARD, cfg.NG

    src = np.asarray(edge_index[0], dtype=np.int64)
    dst = np.asarray(edge_index[1], dtype=np.int64)
    loops = np.arange(N, dtype=np.int64)
    src = np.concatenate([src, loops])
    dst = np.concatenate([dst, loops])
    E = src.shape[0]

    deg = np.bincount(dst, minlength=N).astype(np.int64)
    degp = np.concatenate([deg, np.ones(NP - N, dtype=np.int64)])

    # ---- degree-balanced node permutation: snake-deal into NB bins --------
    order = np.argsort(-degp, kind="stable")          # nodes by degree desc
    i = np.arange(NP)
    r = i // NB                                       # deal round = slot
    cpos = i % NB
    binid = np.where(r % 2 == 0, cpos, NB - 1 - cpos)
    load = np.bincount(binid, weights=degp[order].astype(np.float64), minlength=NB)
    border = np.argsort(-load, kind="stable")         # bins by load desc
    bin_core = np.empty(NB, dtype=np.int64)
    bin_w = np.empty(NB, dtype=np.int64)
    bin_core[border] = np.arange(NB) % N_CORES
    bin_w[border] = np.arange(NB) // N_CORES
    perm_row = np.empty(NP, dtype=np.int64)
    perm_row[order] = bin_core[binid] * SHARD + bin_w[binid] * 128 + r
    inv_perm = np.empty(NP, dtype=np.int64)
    inv_perm[perm_row] = np.arange(NP)

    # permuted per-node arrays
    x = np.asarray(x, dtype=np.float32)
    xp = np.zeros((NP, cfg.F_IN), dtype=np.float32)
    real = inv_perm < N
    xp[real] = x[inv_perm[real]]
    deg_perm = degp[inv_perm].astype(np.float32)      # [NP]

    # x transposed tiles: XT[t, k, kb, p] = xp[t*128+p, kb*128+k]
    KB = cfg.F_IN // 128
    XT = np.ascontiguousarray(
        xp.reshape(NB, 128, KB, 128).transpose(0, 3, 2, 1)
    )  # [NB, 128, KB, 128]
    # 4-tile interleave so one DMA loads 4 tiles contiguously per partition:
    # XT4[q, k, i, kb, p] = XT[4q+i, k, kb, p]
    XT = np.ascontiguousarray(
        XT.reshape(NB // 4, 4, 128, KB, 128).transpose(0, 2, 1, 3, 4)
    )  # [NB/4, 128, 4, KB, 128]
    if USE_BF16:
        XT = XT.astype(mybir.dt.np(BF16))

    # ---- edges -> (core, window, group), sorted ---------------------------
    es = perm_row[src]
    ed = perm_row[dst]
    core_e = ed // SHARD
    w_e = (ed % SHARD) // 128
    slot_e = ed % 128
    grp_e = es >> cfg.QSHIFT
    # sub-split each (w, g) by dst-slot half so every non-first chunk's
    # slots live in one 64-slot block (narrow selection matmuls at legal
    # PE tile positions); slot-sorted within each (core, w, g2)
    NG2 = NG * 2
    g2_e = grp_e * 2 + (slot_e >= 64)
    key = ((core_e * NWC + w_e) * NG2 + g2_e).astype(np.int64)
    eorder = np.argsort(key * 128 + slot_e, kind="stable")
    key_s = key[eorder]
    es_s = es[eorder]
    slot_i = slot_e[eorder]

    counts = np.bincount(key, minlength=N_CORES * NWC * NG2).reshape(
        N_CORES, NWC, NG2
    )
    # shared (all-core) padded sizes per (window, group-half)
    max_cg = counts.max(axis=0)                       # [NWC, NG2]
    n_pad = 128 * ((max_cg + 127) // 128)             # [NWC, NG2] multiple of 128
    cpw_g = n_pad // 128
    cpw_w = cpw_g.sum(axis=1)                         # chunks per window
    tot_cpw = int(cpw_w.sum())
    chunk_off_wg = np.zeros((NWC, NG2), dtype=np.int64)
    chunk_off_w = np.zeros(NWC, dtype=np.int64)
    acc = 0
    for w in range(NWC):
        chunk_off_w[w] = acc
        for g in range(NG2):
            chunk_off_wg[w, g] = acc
            acc += cpw_g[w, g]
    assert acc == tot_cpw

    # per-edge position within its (core, w, g2) run
    gstart = np.zeros(N_CORES * NWC * NG2 + 1, dtype=np.int64)
    np.cumsum(counts.reshape(-1), out=gstart[1:])
    pos = np.arange(E, dtype=np.int64) - gstart[key_s]

    # destination columns in the flat arrays (same offsets on every core)
    wg = key_s % (NWC * NG2)                          # (w, g2) combined
    w_s = wg // NG2
    g_s = wg % NG2
    chunk = chunk_off_wg[w_s, g_s] + pos // 128       # global chunk column
    part = pos % 128
    core_s = key_s // (NWC * NG2)

    TOT_COLS = tot_cpw * 8                            # int16 cols (128 idx -> 8)
    idx_flat = np.zeros((N_CORES, 16, TOT_COLS), dtype=np.int16)
    idx1_flat = np.zeros((N_CORES, 16, TOT_COLS), dtype=np.int16)
    dstloc_flat = np.full((N_CORES, 128, tot_cpw), -1.0, dtype=np.float32)

    # idx position within group = pos; wrapped [16, n/16] at group col offset
    icol = chunk_off_wg[w_s, g_s] * 8 + pos // 16
    ipart = pos % 16
    qbase = (es_s >> cfg.QSHIFT) << cfg.QSHIFT
    lval = es_s - qbase
    ival = lval.astype(np.int16)
    # layer-1 table G1 is stored 4-tile interleaved [NB/4, 128, 4, H1]:
    # physical row of local row l is (l//512)*512 + (l%128)*4 + (l//128)%4
    pval = ((lval >> 9) << 9) + ((lval & 127) << 2) + ((lval >> 7) & 3)
    idx_flat[core_s, ipart, icol] = ival
    idx1_flat[core_s, ipart, icol] = pval.astype(np.int16)

    # dstloc holds slots relative to the chunk's 64-slot half; the
    # aggregation accumulates each half into its own 64-partition PSUM
    # tile (matmul PSUM writes must start at partition 0)
    chunk_half = np.zeros(tot_cpw, dtype=np.int64)
    for w in range(NWC):
        for g in range(NG2):
            a = chunk_off_wg[w, g]
            chunk_half[a : a + cpw_g[w, g]] = g % 2
        # both halves must appear so each Z half gets a start=True matmul
        halves = chunk_half[chunk_off_w[w] : chunk_off_w[w] + cpw_w[w]]
        assert halves.min() == 0 and halves.max() == 1, f"window {w}"
    dstloc_flat[core_s, part, chunk] = (
        slot_i - 64 * chunk_half[chunk]).astype(np.float32)
    idx_flat = np.tile(idx_flat, (1, 8, 1))           # replicate to 128 partitions
    idx1_flat = np.tile(idx1_flat, (1, 8, 1))

    # per-core deg arrays
    deg_shard = np.empty((N_CORES, 128, NWC), dtype=np.float32)
    degT_row = np.empty((N_CORES, 1, SHARD), dtype=np.float32)
    deg_full = np.ascontiguousarray(
        deg_perm.reshape(NB, 128).T
    )  # [128, NB] col t = tile t
    for c in range(N_CORES):
        sh = deg_perm[c * SHARD : (c + 1) * SHARD]
        deg_shard[c] = sh.reshape(NWC, 128).T
        degT_row[c, 0] = sh

    iota = np.broadcast_to(np.arange(128, dtype=np.float32), (128, 128)).copy()

    meta = dict(
        cpw_g=cpw_g, cpw_w=cpw_w, chunk_off_wg=chunk_off_wg,
        chunk_off_w=chunk_off_w, tot_cpw=tot_cpw, tot_cols=TOT_COLS,
        chunk_half=chunk_half,
    )
    host = dict(
        XT=XT, deg_full=deg_full, iota=iota,
        idx_flat=idx_flat, idx1_flat=idx1_flat, dstloc_flat=dstloc_flat,
        deg_shard=deg_shard, degT_row=degT_row,
        inv_perm=inv_perm, perm_row=perm_row,
    )
    return host, meta


# --------------------------------------------------------------------------
# device program
# --------------------------------------------------------------------------
def _build_program(cfg, meta):
    NB, NWC, SHARD = cfg.NB, cfg.NWC, cfg.SHARD
    F_IN, H1, H2, C = cfg.F_IN, cfg.H1, cfg.H2, cfg.C
    KB = F_IN // 128
    KB2 = H1 // 128
    NG = cfg.NG
    cpw_g = meta["cpw_g"]
    cpw_w = meta["cpw_w"]
    chunk_off_wg = meta["chunk_off_wg"]
    chunk_off_w = meta["chunk_off_w"]
    TOT_CPW = meta["tot_cpw"]
    TOT_COLS = meta["tot_cols"]
    MAXCPW = int(cpw_w.max())
    CHUNK_HALF = meta["chunk_half"]

    nc = bacc.Bacc("TRN2", target_bir_lowering=False, debug=False,
                   num_devices=N_CORES)

    WDT = BF16 if USE_BF16 else F32
    # ---- I/O ---------------------------------------------------------------
    XT = nc.dram_tensor("XT", [NB // 4, 128, 4, KB, 128], WDT,
                        kind="ExternalInput")
    W1 = nc.dram_tensor("W1", [128, KB, H1], WDT, kind="ExternalInput")
    W2 = nc.dram_tensor("W2", [128, KB2, H2], F32, kind="ExternalInput")
    W3 = nc.dram_tensor("W3", [H2, 16], F32, kind="ExternalInput")
    B1 = nc.dram_tensor("B1", [1, H1], F32, kind="ExternalInput")
    B2 = nc.dram_tensor("B2", [1, H2], F32, kind="ExternalInput")
    B3 = nc.dram_tensor("B3", [1, 16], F32, kind="ExternalInput")
    IOTA = nc.dram_tensor("IOTA", [128, 128], F32, kind="ExternalInput")
    DEGF = nc.dram_tensor("DEGF", [128, NB], F32, kind="ExternalInput")
    DEGS = nc.dram_tensor("DEGS", [128, NWC], F32, kind="ExternalInput")
    IDX = nc.dram_tensor("IDX", [128, TOT_COLS], I16, kind="ExternalInput")
    IDX1 = nc.dram_tensor("IDX1", [128, TOT_COLS], I16, kind="ExternalInput")
    DSTL = nc.dram_tensor("DSTL", [128, TOT_CPW], F32, kind="ExternalInput")
    OUT = nc.dram_tensor("OUT", [SHARD, C], F32, kind="ExternalOutput")

    # ---- internal DRAM -----------------------------------------------------
    G1 = nc.dram_tensor("G1", [NB // 4, 128, 4, H1], WDT)
    G2S = nc.dram_tensor("G2S", [SHARD, H2], F32)
    G2F = nc.dram_tensor("G2F", [N_CORES, SHARD, H2], F32, addr_space="Shared")
    G3S = nc.dram_tensor("G3S", [SHARD, H2], F32)
    G3F = nc.dram_tensor("G3F", [N_CORES, SHARD, H2], F32, addr_space="Shared")

    rg = [list(range(N_CORES))]

    with tile.TileContext(nc) as tc:
        # ---------- resident constants ----------
        with tc.tile_pool(name="const", bufs=1) as cp:
            identf = cp.tile([128, 128], F32)
            make_identity(nc, identf[:])
            ident = cp.tile([128, 128], F32R)
            nc.vector.tensor_copy(out=ident[:], in_=identf[:])
            iota = cp.tile([128, 128], F32)
            nc.sync.dma_start(out=iota[:], in_=IOTA[:])
            RT = BF16 if USE_BF16 else F32R
            w1 = cp.tile([128, KB, H1], RT)
            nc.sync.dma_start(
                out=w1[:], in_=W1[:] if USE_BF16 else W1[:].bitcast(F32R))
            w2 = cp.tile([128, KB2, H2], F32R)
            nc.sync.dma_start(out=w2[:], in_=W2[:].bitcast(F32R))
            w3 = cp.tile([H2, 16], F32R)
            nc.sync.dma_start(out=w3[:], in_=W3[:].bitcast(F32R))
            b1 = cp.tile([1, H1], F32R)
            nc.sync.dma_start(out=b1[:], in_=B1[:].bitcast(F32R))
            b2 = cp.tile([1, H2], F32R)
            nc.sync.dma_start(out=b2[:], in_=B2[:].bitcast(F32R))
            b3 = cp.tile([1, 16], F32R)
            nc.sync.dma_start(out=b3[:], in_=B3[:].bitcast(F32R))

            degf = cp.tile([128, NB], F32)
            nc.sync.dma_start(out=degf[:], in_=DEGF[:])
            sqf = cp.tile([128, NB], F32)
            nc.scalar.sqrt(out=sqf[:], in_=degf[:])
            dinvf = cp.tile([128, NB], F32)
            nc.vector.reciprocal(out=dinvf[:], in_=sqf[:])

            degs = cp.tile([128, NWC], F32)
            nc.sync.dma_start(out=degs[:], in_=DEGS[:])
            sqs = cp.tile([128, NWC], F32)
            nc.scalar.sqrt(out=sqs[:], in_=degs[:])
            dinvs = cp.tile([128, NWC], F32)
            nc.vector.reciprocal(out=dinvs[:], in_=sqs[:])
            deginvs = cp.tile([128, NWC], F32)
            nc.vector.reciprocal(out=deginvs[:], in_=degs[:])
            # sqrt(deg) column form; per-window rows made via PE transpose
            sq_colr = cp.tile([128, NWC], F32R)
            nc.vector.tensor_copy(out=sq_colr[:], in_=sqs[:])

            dstl = cp.tile([128, TOT_CPW], F32)
            nc.sync.dma_start(out=dstl[:], in_=DSTL[:])
            # flat-layout edge indices resident for phases E and G
            idxfull = cp.tile([128, TOT_COLS], I16)
            nc.sync.dma_start(out=idxfull[:], in_=IDX[:])

            # phase-G softmax staging (batched ln => one act-table switch)
            TTA = cp.tile([128, NWC, C], F32)
            SSA = cp.tile([128, NWC], F32)
            LSA = cp.tile([128, NWC], F32)
            OOA = cp.tile([128, NWC, C], F32)

            # ---------- phase B: table1 = dinv * (x @ W1), all rows ----------
            # 4 tiles per DMA in and out (XT and G1 are 4-tile interleaved)
            with tc.tile_pool(name="l1", bufs=3) as l1p, \
                 tc.tile_pool(name="l1ps", bufs=4, space="PSUM") as l1ps:
                for t4 in range(NB // 4 if "B" in PHASES else 0):
                    if USE_BF16:
                        xtr = l1p.tile([128, 4, KB, 128], BF16, tag="xt")
                        nc.sync.dma_start(out=xtr[:], in_=XT[t4])
                    else:
                        xt = l1p.tile([128, 4, KB, 128], F32, tag="xt")
                        nc.sync.dma_start(out=xt[:], in_=XT[t4])
                        xtr = l1p.tile([128, 4, KB, 128], F32R, tag="xtr")
                        nc.vector.tensor_copy(out=xtr[:], in_=xt[:])
                    g1q = l1p.tile([128, 4, H1], RT, tag="g1q")
                    for i in range(4):
                        t = t4 * 4 + i
                        ps = l1ps.tile([128, H1], F32, space="PSUM", tag="ps")
                        for kb in range(KB):
                            nc.tensor.matmul(out=ps[:], lhsT=xtr[:, i, kb, :],
                                             rhs=w1[:, kb, :],
                                             start=(kb == 0),
                                             stop=(kb == KB - 1))
                        nc.scalar.activation(
                            out=g1q[:, i, :], in_=ps[:],
                            func=mybir.ActivationFunctionType.Copy,
                            scale=dinvf[:, t : t + 1])
                    g1dst = G1[t4]
                    nc.sync.dma_start(
                        out=g1dst if USE_BF16 else g1dst.bitcast(F32R),
                        in_=g1q[:])

            # ---------- per-layer aggregation ----------
            def aggregate(w, tview, fdim, agp, agps, tag, stop_last=False,
                          mdt=F32R, idx_dram=None):
                """Accumulate S^T @ msg for window w into a PSUM tile [128, fdim]."""
                cpw = int(cpw_w[w])
                coff = int(chunk_off_w[w])
                if idx_dram is not None:
                    idxt = agp.tile([128, MAXCPW * 8], I16, tag=tag + "idx")
                    nc.sync.dma_start(
                        out=idxt[:, : cpw * 8],
                        in_=idx_dram[:, coff * 8 : (coff + cpw) * 8])

                    def iap(a, b):
                        return idxt[:, (a - coff) * 8 : (b - coff) * 8]
                else:
                    def iap(a, b):
                        return idxfull[:, a * 8 : b * 8]
                msg = agp.tile([128, MAXCPW, fdim], mdt, tag=tag + "msg")
                for g in range(NG * 2):
                    cg = int(cpw_g[w, g])
                    if cg == 0:
                        continue
                    goff = int(chunk_off_wg[w, g]) - coff
                    qlo = (g >> 1) << cfg.QSHIFT
                    qhi = min(qlo + cfg.QROWS, cfg.NP)
                    step = GATHER_MAX_CHUNKS if GATHER_MAX_CHUNKS else cg
                    tbl = tview(qlo, qhi)
                    if tbl.dtype != mdt:
                        tbl = tbl.bitcast(mdt)
                    gc0 = int(chunk_off_wg[w, g])
                    for c0 in range(0, cg, step):
                        cn = min(step, cg - c0)
                        nc.gpsimd.dma_gather(
                            out_ap=msg[:, goff + c0 : goff + c0 + cn, :],
                            in_ap=tbl,
                            idxs_ap=iap(gc0 + c0, gc0 + c0 + cn),
                            num_idxs=cn * 128,
                            num_idxs_reg=cn * 128,
                            elem_size=fdim,
                            single_packet=SINGLE_PACKET,
                        )
                sdt = mdt if mdt is BF16 else F32R
                # 64-wide S: every chunk's slots live in one 64-slot half
                # (relative slots), accumulated into a per-half PSUM tile
                S = agp.tile([128, MAXCPW, 64], sdt, tag=tag + "S")
                nc.vector.tensor_tensor(
                    out=S[:, :cpw, :],
                    in0=dstl[:, coff : coff + cpw].unsqueeze(2)
                        .to_broadcast([128, cpw, 64]),
                    in1=iota[:, :64].unsqueeze(1).to_broadcast([128, cpw, 64]),
                    op=mybir.AluOpType.is_equal)
                halves = [int(CHUNK_HALF[coff + j]) for j in range(cpw)]
                first = {0: halves.index(0), 1: halves.index(1)}
                last = {0: cpw - 1 - halves[::-1].index(0),
                        1: cpw - 1 - halves[::-1].index(1)}
                Z0 = agps.tile([64, fdim], F32, space="PSUM", tag=tag + "Z0")
                Z1 = agps.tile([64, fdim], F32, space="PSUM", tag=tag + "Z1")
                for j in range(cpw):
                    h = halves[j]
                    nc.tensor.matmul(out=(Z1 if h else Z0)[:],
                                     lhsT=S[:, j, :], rhs=msg[:, j, :],
                                     start=(j == first[h]),
                                     stop=(stop_last and j == last[h]))
                return Z0, Z1

            def sqrtdeg_row(w, agp, sqps, tag):
                """sqrt(deg) of window w as a [1, 128] f32r row (PE transpose)."""
                pt = sqps.tile([1, 128], F32R, space="PSUM", tag=tag + "sqT")
                nc.tensor.transpose(out=pt[:], in_=sq_colr[:, w : w + 1],
                                    identity=ident[:])
                row = agp.tile([1, 128], F32R, tag=tag + "sqr")
                nc.vector.tensor_copy(out=row[:], in_=pt[:])
                return row

            # ---------- phase C: layer-1 aggregation -> table2 shard ----------
            with tc.tile_pool(name="ag1", bufs=2) as agp, \
                 tc.tile_pool(name="ag1z", bufs=2, space="PSUM") as agps, \
                 tc.tile_pool(name="ag1q", bufs=1, space="PSUM") as sqps, \
                 tc.tile_pool(name="ag1g", bufs=1, space="PSUM") as g2ps, \
                 tc.tile_pool(name="ag1t", bufs=2, space="PSUM") as trps:
                def g1view(qlo, qhi):
                    return G1[qlo // 512 : qhi // 512].flatten_outer_dims()

                for w in range(NWC if "C" in PHASES else 0):
                    Z0, Z1 = aggregate(w, g1view, H1, agp, agps, "c",
                                       mdt=BF16 if USE_BF16 else F32R,
                                       idx_dram=IDX1)
                    sqrow = sqrtdeg_row(w, agp, sqps, "c")
                    nc.tensor.matmul(out=Z0[:], lhsT=sqrow[:, 0:64],
                                     rhs=b1[:], start=False, stop=True)
                    nc.tensor.matmul(out=Z1[:], lhsT=sqrow[:, 64:128],
                                     rhs=b1[:], start=False, stop=True)
                    h2 = agp.tile([128, H1], F32R, tag="ch2")
                    nc.scalar.activation(out=h2[0:64, :], in_=Z0[:],
                                         func=mybir.ActivationFunctionType.Relu,
                                         scale=dinvs[0:64, w : w + 1])
                    nc.scalar.activation(out=h2[64:128, :], in_=Z1[:],
                                         func=mybir.ActivationFunctionType.Relu,
                                         scale=dinvs[64:128, w : w + 1])
                    h2T = agp.tile([128, KB2, 128], F32R, tag="ch2T")
                    for kb in range(KB2):
                        tp = trps.tile([128, 128], F32R, space="PSUM", tag="ctp")
                        nc.tensor.transpose(
                            out=tp[:], in_=h2[:, kb * 128 : (kb + 1) * 128],
                            identity=ident[:])
                        nc.vector.tensor_copy(out=h2T[:, kb, :], in_=tp[:])
                    g2p = g2ps.tile([128, H2], F32, space="PSUM", tag="cg2p")
                    for kb in range(KB2):
                        nc.tensor.matmul(out=g2p[:], lhsT=h2T[:, kb, :],
                                         rhs=w2[:, kb, :],
                                         start=(kb == 0), stop=(kb == KB2 - 1))
                    g2sb = agp.tile([128, H2], F32R, tag="cg2sb")
                    nc.vector.tensor_scalar(
                        out=g2sb[:], in0=g2p[:],
                        scalar1=dinvs[:, w : w + 1], scalar2=None,
                        op0=mybir.AluOpType.mult)
                    nc.sync.dma_start(
                        out=G2S[w * 128 : (w + 1) * 128, :].bitcast(F32R),
                        in_=g2sb[:])

            # ---------- phase D: AllGather table2 (chunked: overlaps C tail) --
            NCHUNK = 4
            CQ = SHARD // NCHUNK
            if "D" in PHASES:
                for k in range(NCHUNK):
                    nc.gpsimd.collective_compute(
                        "AllGather", mybir.AluOpType.bypass, replica_groups=rg,
                        ins=[G2S[k * CQ : (k + 1) * CQ, :].opt()],
                        outs=[G2F[:, k * CQ : (k + 1) * CQ, :].opt()])

            # ---------- phase E: layer-2 aggregation -> table3 shard ----------
            with tc.tile_pool(name="ag2", bufs=2) as agp, \
                 tc.tile_pool(name="ag2z", bufs=2, space="PSUM") as agps, \
                 tc.tile_pool(name="ag2q", bufs=1, space="PSUM") as sqps:
                g2flat = G2F[:].flatten_outer_dims()
                for w in range(NWC if "E" in PHASES else 0):
                    Z0, Z1 = aggregate(w, lambda a, b: g2flat[a:b], H2, agp,
                                       agps, "e")
                    sqrow = sqrtdeg_row(w, agp, sqps, "e")
                    nc.tensor.matmul(out=Z0[:], lhsT=sqrow[:, 0:64],
                                     rhs=b2[:], start=False, stop=True)
                    nc.tensor.matmul(out=Z1[:], lhsT=sqrow[:, 64:128],
                                     rhs=b2[:], start=False, stop=True)
                    h3 = agp.tile([128, H2], F32, tag="eh3")
                    nc.scalar.activation(out=h3[0:64, :], in_=Z0[:],
                                         func=mybir.ActivationFunctionType.Relu)
                    nc.scalar.activation(out=h3[64:128, :], in_=Z1[:],
                                         func=mybir.ActivationFunctionType.Relu)
                    g3sb = agp.tile([128, H2], F32R, tag="eg3sb")
                    nc.vector.tensor_scalar(
                        out=g3sb[:], in0=h3[:],
                        scalar1=deginvs[:, w : w + 1], scalar2=None,
                        op0=mybir.AluOpType.mult)
                    nc.sync.dma_start(
                        out=G3S[w * 128 : (w + 1) * 128, :].bitcast(F32R),
                        in_=g3sb[:])

            # ---------- phase F: AllGather table3 (chunked: overlaps E tail) --
            if "F" in PHASES:
                for k in range(NCHUNK):
                    nc.gpsimd.collective_compute(
                        "AllGather", mybir.AluOpType.bypass, replica_groups=rg,
                        ins=[G3S[k * CQ : (k + 1) * CQ, :].opt()],
                        outs=[G3F[:, k * CQ : (k + 1) * CQ, :].opt()])

            # ---------- phase G: layer-3 aggregation + W3 + log_softmax ------
            with tc.tile_pool(name="ag3", bufs=2) as agp, \
                 tc.tile_pool(name="ag3z", bufs=2, space="PSUM") as agps, \
                 tc.tile_pool(name="ag3q", bufs=1, space="PSUM") as sqps, \
                 tc.tile_pool(name="ag3p", bufs=1, space="PSUM") as p3ps, \
                 tc.tile_pool(name="ag3t", bufs=2, space="PSUM") as trps:
                g3flat = G3F[:].flatten_outer_dims()
                for w in range(NWC if "G" in PHASES else 0):
                    Z0, Z1 = aggregate(w, lambda a, b: g3flat[a:b], H2, agp,
                                       agps, "g", stop_last=True)
                    z3 = agp.tile([128, H2], F32R, tag="gz3")
                    nc.scalar.activation(out=z3[0:64, :], in_=Z0[:],
                                         func=mybir.ActivationFunctionType.Copy)
                    nc.scalar.activation(out=z3[64:128, :], in_=Z1[:],
                                         func=mybir.ActivationFunctionType.Copy)
                    tp = trps.tile([H2, 128], F32R, space="PSUM", tag="gtp")
                    nc.tensor.transpose(out=tp[:], in_=z3[:], identity=ident[:])
                    z3T = agp.tile([H2, 128], F32R, tag="gz3T")
                    nc.vector.tensor_copy(out=z3T[:], in_=tp[:])
                    p3 = p3ps.tile([128, 16], F32, space="PSUM", tag="gp3")
                    nc.tensor.matmul(out=p3[:], lhsT=z3T[:], rhs=w3[:],
                                     start=True, stop=False)
                    sqrow = sqrtdeg_row(w, agp, sqps, "g")
                    nc.tensor.matmul(out=p3[:], lhsT=sqrow[:], rhs=b3[:],
                                     start=False, stop=True)
                    zf = agp.tile([128, 16], F32, tag="gzf")
                    nc.vector.tensor_scalar(
                        out=zf[:], in0=p3[:],
                        scalar1=dinvs[:, w : w + 1], scalar2=None,
                        op0=mybir.AluOpType.mult)
                    m = agp.tile([128, 1], F32, tag="gm")
                    nc.vector.reduce_max(out=m[:], in_=zf[:, :C],
                                         axis=mybir.AxisListType.X)
                    nc.vector.tensor_scalar(
                        out=TTA[:, w, :], in0=zf[:, :C], scalar1=m[:],
                        scalar2=None, op0=mybir.AluOpType.subtract)
                    ee = agp.tile([128, C], F32, tag="gee")
                    nc.scalar.activation(out=ee[:], in_=TTA[:, w, :],
                                         func=mybir.ActivationFunctionType.Exp,
                                         accum_out=SSA[:, w : w + 1])
                # single ln pass over all windows' exp-sums (one table load),
                # then subtract and store
                if "G" in PHASES:
                    nc.scalar.activation(out=LSA[:], in_=SSA[:],
                                         func=mybir.ActivationFunctionType.Ln)
                    nc.vector.tensor_tensor(
                        out=OOA[:],
                        in0=TTA[:],
                        in1=LSA[:].unsqueeze(2).to_broadcast([128, NWC, C]),
                        op=mybir.AluOpType.subtract)
                    for w in range(NWC):
                        nc.sync.dma_start(out=OUT[w * 128 : (w + 1) * 128, :],
                                          in_=OOA[:, w, :])

        if DEBUG_TAPS:
            DG1 = nc.dram_tensor("DG1", [cfg.NP, H1], F32, kind="ExternalOutput")
            DG2 = nc.dram_tensor("DG2", [SHARD, H2], F32, kind="ExternalOutput")
            DG3 = nc.dram_tensor("DG3", [SHARD, H2], F32, kind="ExternalOutput")
            nc.sync.dma_start(out=DG1[:], in_=G1[:])
            nc.sync.dma_start(out=DG2[:], in_=G2S[:])
            nc.sync.dma_start(out=DG3[:], in_=G3S[:])

    nc.compile()
    return nc


# --------------------------------------------------------------------------
# driver
# --------------------------------------------------------------------------
def _weights_layout(W1, W2, W3, b1, b2, b3, cfg):
    KB = cfg.F_IN // 128
    KB2 = cfg.H1 // 128
    W1t = np.ascontiguousarray(
        np.asarray(W1, np.float32).reshape(KB, 128, cfg.H1).transpose(1, 0, 2))
    W2t = np.ascontiguousarray(
        np.asarray(W2, np.float32).reshape(KB2, 128, cfg.H2).transpose(1, 0, 2))
    W3p = np.zeros((cfg.H2, 16), np.float32)
    W3p[:, : cfg.C] = np.asarray(W3, np.float32)
    B1 = np.asarray(b1, np.float32).reshape(1, cfg.H1)
    B2 = np.asarray(b2, np.float32).reshape(1, cfg.H2)
    B3 = np.zeros((1, 16), np.float32)
    B3[0, : cfg.C] = np.asarray(b3, np.float32)
    if USE_BF16:
        W1t = W1t.astype(mybir.dt.np(BF16))
    return W1t, W2t, W3p, B1, B2, B3


def prepare(x, edge_index, W1, b1, W2, b2, W3, b3, cfg=FULL_CFG):
    host, meta = _preprocess(x, edge_index, cfg)
    W1t, W2t, W3p, B1, B2, B3 = _weights_layout(W1, W2, W3, b1, b2, b3, cfg)
    nc = _build_program(cfg, meta)
    in_maps = []
    for c in range(N_CORES):
        in_maps.append({
            "XT": host["XT"], "W1": W1t, "W2": W2t, "W3": W3p,
            "B1": B1, "B2": B2, "B3": B3,
            "IOTA": host["iota"], "DEGF": host["deg_full"],
            "DEGS": host["deg_shard"][c],
            "IDX": host["idx_flat"][c], "IDX1": host["idx1_flat"][c],
            "DSTL": host["dstloc_flat"][c],
        })
    return nc, in_maps, host


def run(x, edge_index, W1, b1, W2, b2, W3, b3, cfg=FULL_CFG, trace=False):
    nc, in_maps, host = prepare(x, edge_index, W1, b1, W2, b2, W3, b3, cfg)
    res = bass_utils.run_bass_kernel_spmd(
        nc, in_maps, core_ids=list(range(N_CORES)), trace=trace)

    outp = np.concatenate([res.results[c]["OUT"] for c in range(N_CORES)], axis=0)
    out = outp[host["perm_row"][: cfg.N]]
    return out.astype(np.float32), res


def make_runner(cfg, nc, in_maps):
    """Build a reusable jitted 8-core runner with inputs resident on device.

    Returns (fn, args) where fn(*args) -> list of per-core OUT arrays; call
    repeatedly for timing without host->device transfer of the big inputs.
    """
    import jax
    from jax.sharding import Mesh, PartitionSpec
    from jax.experimental.shard_map import shard_map
    from concourse import bass2jax as b2j

    b2j.install_neuronx_cc_hook()
    partition_name = nc.partition_id_tensor.name if nc.partition_id_tensor else None
    in_names, out_names, out_avals, zero_outs = [], [], [], []
    for alloc in nc.m.functions[0].allocations:
        if not isinstance(alloc, mybir.MemoryLocationSet):
            continue
        name = alloc.memorylocations[0].name
        if alloc.kind == "ExternalInput":
            if name != partition_name:
                in_names.append(name)
        elif alloc.kind == "ExternalOutput":
            shape = tuple(alloc.tensor_shape)
            dtype = mybir.dt.np(alloc.dtype)
            out_names.append(name)
            out_avals.append(jax.core.ShapedArray(shape, dtype))
            zero_outs.append(np.zeros(shape, dtype))
    n_params = len(in_names)
    all_in_names = list(in_names) + out_names
    if partition_name is not None:
        all_in_names.append(partition_name)

    def _body(*args):
        operands = list(args)
        if partition_name is not None:
            operands.append(b2j.partition_id_tensor())
        outs = b2j._bass_exec_p.bind(
            *operands, out_avals=tuple(out_avals), in_names=tuple(all_in_names),
            out_names=tuple(out_names), lowering_input_output_aliases=(),
            sim_require_finite=False, sim_require_nnan=False, nc=nc)
        return tuple(outs)

    devices = jax.devices()[:N_CORES]
    mesh = Mesh(np.asarray(devices), ("core",))
    in_specs = (PartitionSpec("core"),) * (n_params + len(out_names))
    out_specs = (PartitionSpec("core"),) * len(out_names)
    sharded = jax.jit(
        shard_map(_body, mesh=mesh, in_specs=in_specs, out_specs=out_specs,
                  check_rep=False),
        keep_unused=True)
    sh = jax.sharding.NamedSharding(mesh, PartitionSpec("core"))
    concat_in = [
        jax.device_put(
            np.concatenate([np.asarray(in_maps[c][k]) for c in range(N_CORES)], 0),
            sh)
        for k in in_names
    ]
    concat_zeros = [
        jax.device_put(np.zeros((N_CORES * z.shape[0], *z.shape[1:]), z.dtype), sh)
        for z in zero_outs
    ]
    args = concat_in + concat_zeros

    def fn(*a):
        outs = sharded(*a)
        return {name: outs[i] for i, name in enumerate(out_names)}

    return fn, args, out_names


def kernel(**inputs):
    out, _ = run(
        inputs["x"], inputs["edge_index"],
        inputs["W1"], inputs["b1"], inputs["W2"], inputs["b2"],
        inputs["W3"], inputs["b3"], cfg=FULL_CFG)
    return out



# revision 72
# speedup vs baseline: 1.1255x; 1.0432x over previous
"""GCN (3-layer, symmetric-normalized) on 8 Trainium2 NeuronCores.

Strategy
--------
z_l = A_n @ (h_l W_l) + b_l with A_n = D^-1/2 (A+I) D^-1/2.  We factor the
edge norm into the node tables:  table_l = dinv * (h_l W_l)  (rows scaled by
dinv[src]), aggregate with a 0/1 selection matmul per 128-dst window
(S^T @ msg accumulated in PSUM), and apply dinv[dst] afterwards.  The bias is
injected into PSUM as a rank-1 matmul sqrt(deg) x b so that the final scale
dinv * (Z + sqrtdeg x b) = dinv*Z + b.

Sharding: dst nodes are permuted (degree-balanced bins) into 8 x 98 windows of
128; each core owns 98 windows and all edges targeting them.  Layer-1 table is
computed redundantly on every core (cheaper than an AllGather of 100MB);
layer-2/3 tables are computed shard-wise and AllGathered (25MB).

Gathers use dma_gather (int16 indices, 16-partition-wrapped, replicated x8)
in single-packet mode, capped at 7 chunks (896 idxs = 56 descs/lane; the HW
packet limit is 64 descs).  The table is split into 4 row-groups of 32768 so
indices fit int16; the layer-1 path (x, W1, table, messages, S) runs in bf16,
layers 2/3 in float32r.  Edges are further split by dst-slot half and
slot-sorted, so selection matrices are 64 wide and each half accumulates into
its own 64-partition PSUM tile.  Phase G stages softmax terms for all windows,
then does one batched ln (avoids per-window activation-table reloads).
"""

import math

import numpy as np

try:
    import concourse  # noqa: F401
except ImportError:  # pragma: no cover
    import sys

    sys.path.insert(0, "/opt/trn_rl_repo")

import concourse.bass as bass
import concourse.bacc as bacc
import concourse.mybir as mybir
import concourse.tile as tile
from concourse import bass_utils
from concourse.masks import make_identity

F32 = mybir.dt.float32
F32R = mybir.dt.float32r
BF16 = mybir.dt.bfloat16
I16 = mybir.dt.int16

import os

N_CORES = 8
DEBUG_TAPS = False
PHASES = os.environ.get("K_PHASES", "BCDEFG")  # debug: which phases to build
# single_packet=True concatenates each DMA lane's descriptors into one packet;
# HW caps a packet at 64 descriptors, so it hangs when num_idxs/16 + 1 > 64.
# Keep num_idxs <= 7*128 = 896 (56 descs/lane) per gather call and it is safe.
SINGLE_PACKET = os.environ.get("K_SINGLE_PACKET", "1") == "1"
GATHER_MAX_CHUNKS = int(os.environ.get("K_GATHER_MAX_CHUNKS", "7"))
# bf16 for x/W1/layer-1 table: halves gather + input DMA traffic.
USE_BF16 = os.environ.get("K_BF16", "1") == "1"


class Cfg:
    def __init__(self, N, NB, F_IN, H1, H2, C, QSHIFT):
        assert NB % N_CORES == 0
        self.N = N                    # real nodes
        self.NB = NB                  # total 128-node windows (bins)
        self.NP = NB * 128            # padded nodes
        self.NWC = NB // N_CORES      # windows per core
        self.SHARD = self.NWC * 128   # rows per core
        self.F_IN = F_IN              # input features (mult of 128)
        self.H1 = H1                  # layer-1 width (mult of 128)
        self.H2 = H2                  # layer-2 width (<=128)
        self.C = C                    # classes (<=16)
        self.QSHIFT = QSHIFT          # group shift (rows per group = 1<<QSHIFT)
        self.QROWS = 1 << QSHIFT
        self.NG = (self.NP + self.QROWS - 1) >> QSHIFT
        assert F_IN % 128 == 0 and H1 % 128 == 0 and H2 <= 128 and C <= 16


FULL_CFG = Cfg(N=100000, NB=784, F_IN=512, H1=256, H2=64, C=10, QSHIFT=15)


# --------------------------------------------------------------------------
# host-side graph preprocessing
# --------------------------------------------------------------------------
def _preprocess(x, edge_index, cfg):
    N, NP, NB = cfg.N, cfg.NP, cfg.NB
    NWC, SHARD, NG = cfg.NWC, cfg.SHARD, cfg.NG

    src = np.asarray(edge_index[0], dtype=np.int64)
    dst = np.asarray(edge_index[1], dtype=np.int64)
    loops = np.arange(N, dtype=np.int64)
    src = np.concatenate([src, loops])
    dst = np.concatenate([dst, loops])
    E = src.shape[0]

    deg = np.bincount(dst, minlength=N).astype(np.int64)
    degp = np.concatenate([deg, np.ones(NP - N, dtype=np.int64)])

    # ---- degree-balanced node permutation: snake-deal into NB bins --------
    order = np.argsort(-degp, kind="stable")          # nodes by degree desc
    i = np.arange(NP)
    r = i // NB                                       # deal round = slot
    cpos = i % NB
    binid = np.where(r % 2 == 0, cpos, NB - 1 - cpos)
    load = np.bincount(binid, weights=degp[order].astype(np.float64), minlength=NB)
    border = np.argsort(-load, kind="stable")         # bins by load desc
    bin_core = np.empty(NB, dtype=np.int64)
    bin_w = np.empty(NB, dtype=np.int64)
    bin_core[border] = np.arange(NB) % N_CORES
    bin_w[border] = np.arange(NB) // N_CORES
    perm_row = np.empty(NP, dtype=np.int64)
    perm_row[order] = bin_core[binid] * SHARD + bin_w[binid] * 128 + r
    inv_perm = np.empty(NP, dtype=np.int64)
    inv_perm[perm_row] = np.arange(NP)

    # permuted per-node arrays
    x = np.asarray(x, dtype=np.float32)
    xp = np.zeros((NP, cfg.F_IN), dtype=np.float32)
    real = inv_perm < N
    xp[real] = x[inv_perm[real]]
    deg_perm = degp[inv_perm].astype(np.float32)      # [NP]

    # x transposed tiles: XT[t, k, kb, p] = xp[t*128+p, kb*128+k]
    KB = cfg.F_IN // 128
    XT = np.ascontiguousarray(
        xp.reshape(NB, 128, KB, 128).transpose(0, 3, 2, 1)
    )  # [NB, 128, KB, 128]
    # 4-tile interleave so one DMA loads 4 tiles contiguously per partition:
    # XT4[q, k, i, kb, p] = XT[4q+i, k, kb, p]
    XT = np.ascontiguousarray(
        XT.reshape(NB // 4, 4, 128, KB, 128).transpose(0, 2, 1, 3, 4)
    )  # [NB/4, 128, 4, KB, 128]
    if USE_BF16:
        XT = XT.astype(mybir.dt.np(BF16))

    # ---- edges -> (core, window, group), sorted ---------------------------
    es = perm_row[src]
    ed = perm_row[dst]
    core_e = ed // SHARD
    w_e = (ed % SHARD) // 128
    slot_e = ed % 128
    grp_e = es >> cfg.QSHIFT
    # sub-split each (w, g) by dst-slot half so every non-first chunk's
    # slots live in one 64-slot block (narrow selection matmuls at legal
    # PE tile positions); slot-sorted within each (core, w, g2)
    NG2 = NG * 2
    g2_e = grp_e * 2 + (slot_e >= 64)
    key = ((core_e * NWC + w_e) * NG2 + g2_e).astype(np.int64)
    eorder = np.argsort(key * 128 + slot_e, kind="stable")
    key_s = key[eorder]
    es_s = es[eorder]
    slot_i = slot_e[eorder]

    counts = np.bincount(key, minlength=N_CORES * NWC * NG2).reshape(
        N_CORES, NWC, NG2
    )
    # shared (all-core) padded sizes per (window, group-half)
    max_cg = counts.max(axis=0)                       # [NWC, NG2]
    n_pad = 128 * ((max_cg + 127) // 128)             # [NWC, NG2] multiple of 128
    cpw_g = n_pad // 128
    cpw_w = cpw_g.sum(axis=1)                         # chunks per window
    tot_cpw = int(cpw_w.sum())
    chunk_off_wg = np.zeros((NWC, NG2), dtype=np.int64)
    chunk_off_w = np.zeros(NWC, dtype=np.int64)
    acc = 0
    for w in range(NWC):
        chunk_off_w[w] = acc
        for g in range(NG2):
            chunk_off_wg[w, g] = acc
            acc += cpw_g[w, g]
    assert acc == tot_cpw

    # per-edge position within its (core, w, g2) run
    gstart = np.zeros(N_CORES * NWC * NG2 + 1, dtype=np.int64)
    np.cumsum(counts.reshape(-1), out=gstart[1:])
    pos = np.arange(E, dtype=np.int64) - gstart[key_s]

    # destination columns in the flat arrays (same offsets on every core)
    wg = key_s % (NWC * NG2)                          # (w, g2) combined
    w_s = wg // NG2
    g_s = wg % NG2
    chunk = chunk_off_wg[w_s, g_s] + pos // 128       # global chunk column
    part = pos % 128
    core_s = key_s // (NWC * NG2)

    TOT_COLS = tot_cpw * 8                            # int16 cols (128 idx -> 8)
    idx_flat = np.zeros((N_CORES, 16, TOT_COLS), dtype=np.int16)
    idx1_flat = np.zeros((N_CORES, 16, TOT_COLS), dtype=np.int16)
    dstloc_flat = np.full((N_CORES, 128, tot_cpw), -1.0, dtype=np.float32)

    # idx position within group = pos; wrapped [16, n/16] at group col offset
    icol = chunk_off_wg[w_s, g_s] * 8 + pos // 16
    ipart = pos % 16
    qbase = (es_s >> cfg.QSHIFT) << cfg.QSHIFT
    lval = es_s - qbase
    ival = lval.astype(np.int16)
    # layer-1 table G1 is stored 4-tile interleaved [NB/4, 128, 4, H1]:
    # physical row of local row l is (l//512)*512 + (l%128)*4 + (l//128)%4
    pval = ((lval >> 9) << 9) + ((lval & 127) << 2) + ((lval >> 7) & 3)
    idx_flat[core_s, ipart, icol] = ival
    idx1_flat[core_s, ipart, icol] = pval.astype(np.int16)

    # dstloc holds slots relative to the chunk's 64-slot half; the
    # aggregation accumulates each half into its own 64-partition PSUM
    # tile (matmul PSUM writes must start at partition 0)
    chunk_half = np.zeros(tot_cpw, dtype=np.int64)
    for w in range(NWC):
        for g in range(NG2):
            a = chunk_off_wg[w, g]
            chunk_half[a : a + cpw_g[w, g]] = g % 2
        # both halves must appear so each Z half gets a start=True matmul
        halves = chunk_half[chunk_off_w[w] : chunk_off_w[w] + cpw_w[w]]
        assert halves.min() == 0 and halves.max() == 1, f"window {w}"
    dstloc_flat[core_s, part, chunk] = (
        slot_i - 64 * chunk_half[chunk]).astype(np.float32)
    idx_flat = np.tile(idx_flat, (1, 8, 1))           # replicate to 128 partitions
    idx1_flat = np.tile(idx1_flat, (1, 8, 1))

    # per-core deg arrays
    deg_shard = np.empty((N_CORES, 128, NWC), dtype=np.float32)
    degT_row = np.empty((N_CORES, 1, SHARD), dtype=np.float32)
    deg_full = np.ascontiguousarray(
        deg_perm.reshape(NB, 128).T
    )  # [128, NB] col t = tile t
    for c in range(N_CORES):
        sh = deg_perm[c * SHARD : (c + 1) * SHARD]
        deg_shard[c] = sh.reshape(NWC, 128).T
        degT_row[c, 0] = sh

    iota = np.broadcast_to(np.arange(128, dtype=np.float32), (128, 128)).copy()

    meta = dict(
        cpw_g=cpw_g, cpw_w=cpw_w, chunk_off_wg=chunk_off_wg,
        chunk_off_w=chunk_off_w, tot_cpw=tot_cpw, tot_cols=TOT_COLS,
        chunk_half=chunk_half,
    )
    host = dict(
        XT=XT, deg_full=deg_full, iota=iota,
        idx_flat=idx_flat, idx1_flat=idx1_flat, dstloc_flat=dstloc_flat,
        deg_shard=deg_shard, degT_row=degT_row,
        inv_perm=inv_perm, perm_row=perm_row,
    )
    return host, meta


# --------------------------------------------------------------------------
# device program
# --------------------------------------------------------------------------
def _build_program(cfg, meta):
    NB, NWC, SHARD = cfg.NB, cfg.NWC, cfg.SHARD
    F_IN, H1, H2, C = cfg.F_IN, cfg.H1, cfg.H2, cfg.C
    KB = F_IN // 128
    KB2 = H1 // 128
    NG = cfg.NG
    cpw_g = meta["cpw_g"]
    cpw_w = meta["cpw_w"]
    chunk_off_wg = meta["chunk_off_wg"]
    chunk_off_w = meta["chunk_off_w"]
    TOT_CPW = meta["tot_cpw"]
    TOT_COLS = meta["tot_cols"]
    MAXCPW = int(cpw_w.max())
    CHUNK_HALF = meta["chunk_half"]

    nc = bacc.Bacc("TRN2", target_bir_lowering=False, debug=False,
                   num_devices=N_CORES)

    WDT = BF16 if USE_BF16 else F32
    # ---- I/O ---------------------------------------------------------------
    XT = nc.dram_tensor("XT", [NB // 4, 128, 4, KB, 128], WDT,
                        kind="ExternalInput")
    W1 = nc.dram_tensor("W1", [128, KB, H1], WDT, kind="ExternalInput")
    W2 = nc.dram_tensor("W2", [128, KB2, H2], F32, kind="ExternalInput")
    W3 = nc.dram_tensor("W3", [H2, 16], F32, kind="ExternalInput")
    B1 = nc.dram_tensor("B1", [1, H1], F32, kind="ExternalInput")
    B2 = nc.dram_tensor("B2", [1, H2], F32, kind="ExternalInput")
    B3 = nc.dram_tensor("B3", [1, 16], F32, kind="ExternalInput")
    IOTA = nc.dram_tensor("IOTA", [128, 128], F32, kind="ExternalInput")
    DEGF = nc.dram_tensor("DEGF", [128, NB], F32, kind="ExternalInput")
    DEGS = nc.dram_tensor("DEGS", [128, NWC], F32, kind="ExternalInput")
    IDX = nc.dram_tensor("IDX", [128, TOT_COLS], I16, kind="ExternalInput")
    IDX1 = nc.dram_tensor("IDX1", [128, TOT_COLS], I16, kind="ExternalInput")
    DSTL = nc.dram_tensor("DSTL", [128, TOT_CPW], F32, kind="ExternalInput")
    OUT = nc.dram_tensor("OUT", [SHARD, C], F32, kind="ExternalOutput")

    # ---- internal DRAM -----------------------------------------------------
    G1 = nc.dram_tensor("G1", [NB // 4, 128, 4, H1], WDT)
    G2S = nc.dram_tensor("G2S", [SHARD, H2], F32)
    G2F = nc.dram_tensor("G2F", [cfg.NP, H2], F32, addr_space="Shared")
    G3S = nc.dram_tensor("G3S", [SHARD, H2], F32)
    G3F = nc.dram_tensor("G3F", [cfg.NP, H2], F32, addr_space="Shared")

    rg = [list(range(N_CORES))]

    with tile.TileContext(nc) as tc:
        # ---------- resident constants ----------
        with tc.tile_pool(name="const", bufs=1) as cp:
            identf = cp.tile([128, 128], F32)
            make_identity(nc, identf[:])
            ident = cp.tile([128, 128], F32R)
            nc.vector.tensor_copy(out=ident[:], in_=identf[:])
            iota = cp.tile([128, 128], F32)
            nc.sync.dma_start(out=iota[:], in_=IOTA[:])
            RT = BF16 if USE_BF16 else F32R
            w1 = cp.tile([128, KB, H1], RT)
            nc.sync.dma_start(
                out=w1[:], in_=W1[:] if USE_BF16 else W1[:].bitcast(F32R))
            w2 = cp.tile([128, KB2, H2], F32R)
            nc.sync.dma_start(out=w2[:], in_=W2[:].bitcast(F32R))
            w3 = cp.tile([H2, 16], F32R)
            nc.sync.dma_start(out=w3[:], in_=W3[:].bitcast(F32R))
            b1 = cp.tile([1, H1], F32R)
            nc.sync.dma_start(out=b1[:], in_=B1[:].bitcast(F32R))
            b2 = cp.tile([1, H2], F32R)
            nc.sync.dma_start(out=b2[:], in_=B2[:].bitcast(F32R))
            b3 = cp.tile([1, 16], F32R)
            nc.sync.dma_start(out=b3[:], in_=B3[:].bitcast(F32R))

            degf = cp.tile([128, NB], F32)
            nc.sync.dma_start(out=degf[:], in_=DEGF[:])
            sqf = cp.tile([128, NB], F32)
            nc.scalar.sqrt(out=sqf[:], in_=degf[:])
            dinvf = cp.tile([128, NB], F32)
            nc.vector.reciprocal(out=dinvf[:], in_=sqf[:])

            degs = cp.tile([128, NWC], F32)
            nc.sync.dma_start(out=degs[:], in_=DEGS[:])
            sqs = cp.tile([128, NWC], F32)
            nc.scalar.sqrt(out=sqs[:], in_=degs[:])
            dinvs = cp.tile([128, NWC], F32)
            nc.vector.reciprocal(out=dinvs[:], in_=sqs[:])
            deginvs = cp.tile([128, NWC], F32)
            nc.vector.reciprocal(out=deginvs[:], in_=degs[:])
            # sqrt(deg) column form; per-window rows made via PE transpose
            sq_colr = cp.tile([128, NWC], F32R)
            nc.vector.tensor_copy(out=sq_colr[:], in_=sqs[:])

            dstl = cp.tile([128, TOT_CPW], F32)
            nc.sync.dma_start(out=dstl[:], in_=DSTL[:])
            # flat-layout edge indices resident for phases E and G
            idxfull = cp.tile([128, TOT_COLS], I16)
            nc.sync.dma_start(out=idxfull[:], in_=IDX[:])

            # phase-G softmax staging (batched ln => one act-table switch)
            TTA = cp.tile([128, NWC, C], F32)
            SSA = cp.tile([128, NWC], F32)
            LSA = cp.tile([128, NWC], F32)
            OOA = cp.tile([128, NWC, C], F32)

            # ---------- phase B: table1 = dinv * (x @ W1), all rows ----------
            # 4 tiles per DMA in and out (XT and G1 are 4-tile interleaved)
            with tc.tile_pool(name="l1", bufs=3) as l1p, \
                 tc.tile_pool(name="l1ps", bufs=4, space="PSUM") as l1ps:
                for t4 in range(NB // 4 if "B" in PHASES else 0):
                    if USE_BF16:
                        xtr = l1p.tile([128, 4, KB, 128], BF16, tag="xt")
                        nc.sync.dma_start(out=xtr[:], in_=XT[t4])
                    else:
                        xt = l1p.tile([128, 4, KB, 128], F32, tag="xt")
                        nc.sync.dma_start(out=xt[:], in_=XT[t4])
                        xtr = l1p.tile([128, 4, KB, 128], F32R, tag="xtr")
                        nc.vector.tensor_copy(out=xtr[:], in_=xt[:])
                    g1q = l1p.tile([128, 4, H1], RT, tag="g1q")
                    for i in range(4):
                        t = t4 * 4 + i
                        ps = l1ps.tile([128, H1], F32, space="PSUM", tag="ps")
                        for kb in range(KB):
                            nc.tensor.matmul(out=ps[:], lhsT=xtr[:, i, kb, :],
                                             rhs=w1[:, kb, :],
                                             start=(kb == 0),
                                             stop=(kb == KB - 1))
                        nc.scalar.activation(
                            out=g1q[:, i, :], in_=ps[:],
                            func=mybir.ActivationFunctionType.Copy,
                            scale=dinvf[:, t : t + 1])
                    g1dst = G1[t4]
                    nc.sync.dma_start(
                        out=g1dst if USE_BF16 else g1dst.bitcast(F32R),
                        in_=g1q[:])

            # ---------- per-layer aggregation ----------
            def aggregate(w, tview, fdim, agp, agps, tag, stop_last=False,
                          mdt=F32R, idx_dram=None):
                """Accumulate S^T @ msg for window w into a PSUM tile [128, fdim]."""
                cpw = int(cpw_w[w])
                coff = int(chunk_off_w[w])
                if idx_dram is not None:
                    idxt = agp.tile([128, MAXCPW * 8], I16, tag=tag + "idx")
                    nc.sync.dma_start(
                        out=idxt[:, : cpw * 8],
                        in_=idx_dram[:, coff * 8 : (coff + cpw) * 8])

                    def iap(a, b):
                        return idxt[:, (a - coff) * 8 : (b - coff) * 8]
                else:
                    def iap(a, b):
                        return idxfull[:, a * 8 : b * 8]
                msg = agp.tile([128, MAXCPW, fdim], mdt, tag=tag + "msg")
                for g in range(NG * 2):
                    cg = int(cpw_g[w, g])
                    if cg == 0:
                        continue
                    goff = int(chunk_off_wg[w, g]) - coff
                    qlo = (g >> 1) << cfg.QSHIFT
                    qhi = min(qlo + cfg.QROWS, cfg.NP)
                    step = GATHER_MAX_CHUNKS if GATHER_MAX_CHUNKS else cg
                    tbl = tview(qlo, qhi)
                    if tbl.dtype != mdt:
                        tbl = tbl.bitcast(mdt)
                    gc0 = int(chunk_off_wg[w, g])
                    for c0 in range(0, cg, step):
                        cn = min(step, cg - c0)
                        nc.gpsimd.dma_gather(
                            out_ap=msg[:, goff + c0 : goff + c0 + cn, :],
                            in_ap=tbl,
                            idxs_ap=iap(gc0 + c0, gc0 + c0 + cn),
                            num_idxs=cn * 128,
                            num_idxs_reg=cn * 128,
                            elem_size=fdim,
                            single_packet=SINGLE_PACKET,
                        )
                sdt = mdt if mdt is BF16 else F32R
                # 64-wide S: every chunk's slots live in one 64-slot half
                # (relative slots), accumulated into a per-half PSUM tile
                S = agp.tile([128, MAXCPW, 64], sdt, tag=tag + "S")
                nc.vector.tensor_tensor(
                    out=S[:, :cpw, :],
                    in0=dstl[:, coff : coff + cpw].unsqueeze(2)
                        .to_broadcast([128, cpw, 64]),
                    in1=iota[:, :64].unsqueeze(1).to_broadcast([128, cpw, 64]),
                    op=mybir.AluOpType.is_equal)
                halves = [int(CHUNK_HALF[coff + j]) for j in range(cpw)]
                first = {0: halves.index(0), 1: halves.index(1)}
                last = {0: cpw - 1 - halves[::-1].index(0),
                        1: cpw - 1 - halves[::-1].index(1)}
                Z0 = agps.tile([64, fdim], F32, space="PSUM", tag=tag + "Z0")
                Z1 = agps.tile([64, fdim], F32, space="PSUM", tag=tag + "Z1")
                for j in range(cpw):
                    h = halves[j]
                    nc.tensor.matmul(out=(Z1 if h else Z0)[:],
                                     lhsT=S[:, j, :], rhs=msg[:, j, :],
                                     start=(j == first[h]),
                                     stop=(stop_last and j == last[h]))
                return Z0, Z1

            def sqrtdeg_row(w, agp, sqps, tag):
                """sqrt(deg) of window w as a [1, 128] f32r row (PE transpose)."""
                pt = sqps.tile([1, 128], F32R, space="PSUM", tag=tag + "sqT")
                nc.tensor.transpose(out=pt[:], in_=sq_colr[:, w : w + 1],
                                    identity=ident[:])
                row = agp.tile([1, 128], F32R, tag=tag + "sqr")
                nc.vector.tensor_copy(out=row[:], in_=pt[:])
                return row

            # ---------- phase C: layer-1 aggregation -> table2 shard ----------
            with tc.tile_pool(name="ag1", bufs=2) as agp, \
                 tc.tile_pool(name="ag1z", bufs=2, space="PSUM") as agps, \
                 tc.tile_pool(name="ag1q", bufs=1, space="PSUM") as sqps, \
                 tc.tile_pool(name="ag1g", bufs=1, space="PSUM") as g2ps, \
                 tc.tile_pool(name="ag1t", bufs=2, space="PSUM") as trps:
                def g1view(qlo, qhi):
                    return G1[qlo // 512 : qhi // 512].flatten_outer_dims()

                for w in range(NWC if "C" in PHASES else 0):
                    Z0, Z1 = aggregate(w, g1view, H1, agp, agps, "c",
                                       mdt=BF16 if USE_BF16 else F32R,
                                       idx_dram=IDX1)
                    sqrow = sqrtdeg_row(w, agp, sqps, "c")
                    nc.tensor.matmul(out=Z0[:], lhsT=sqrow[:, 0:64],
                                     rhs=b1[:], start=False, stop=True)
                    nc.tensor.matmul(out=Z1[:], lhsT=sqrow[:, 64:128],
                                     rhs=b1[:], start=False, stop=True)
                    h2 = agp.tile([128, H1], F32R, tag="ch2")
                    nc.scalar.activation(out=h2[0:64, :], in_=Z0[:],
                                         func=mybir.ActivationFunctionType.Relu,
                                         scale=dinvs[0:64, w : w + 1])
                    nc.scalar.activation(out=h2[64:128, :], in_=Z1[:],
                                         func=mybir.ActivationFunctionType.Relu,
                                         scale=dinvs[64:128, w : w + 1])
                    h2T = agp.tile([128, KB2, 128], F32R, tag="ch2T")
                    for kb in range(KB2):
                        tp = trps.tile([128, 128], F32R, space="PSUM", tag="ctp")
                        nc.tensor.transpose(
                            out=tp[:], in_=h2[:, kb * 128 : (kb + 1) * 128],
                            identity=ident[:])
                        nc.vector.tensor_copy(out=h2T[:, kb, :], in_=tp[:])
                    g2p = g2ps.tile([128, H2], F32, space="PSUM", tag="cg2p")
                    for kb in range(KB2):
                        nc.tensor.matmul(out=g2p[:], lhsT=h2T[:, kb, :],
                                         rhs=w2[:, kb, :],
                                         start=(kb == 0), stop=(kb == KB2 - 1))
                    g2sb = agp.tile([128, H2], F32R, tag="cg2sb")
                    nc.vector.tensor_scalar(
                        out=g2sb[:], in0=g2p[:],
                        scalar1=dinvs[:, w : w + 1], scalar2=None,
                        op0=mybir.AluOpType.mult)
                    nc.sync.dma_start(
                        out=G2S[w * 128 : (w + 1) * 128, :].bitcast(F32R),
                        in_=g2sb[:])

            # ---------- phase D: AllGather table2 ----------
            if "D" in PHASES:
                nc.gpsimd.collective_compute(
                    "AllGather", mybir.AluOpType.bypass, replica_groups=rg,
                    ins=[G2S[:].opt()], outs=[G2F[:].opt()])

            # ---------- phase E: layer-2 aggregation -> table3 shard ----------
            with tc.tile_pool(name="ag2", bufs=2) as agp, \
                 tc.tile_pool(name="ag2z", bufs=2, space="PSUM") as agps, \
                 tc.tile_pool(name="ag2q", bufs=1, space="PSUM") as sqps:
                for w in range(NWC if "E" in PHASES else 0):
                    Z0, Z1 = aggregate(w, lambda a, b: G2F[a:b, :], H2, agp,
                                       agps, "e")
                    sqrow = sqrtdeg_row(w, agp, sqps, "e")
                    nc.tensor.matmul(out=Z0[:], lhsT=sqrow[:, 0:64],
                                     rhs=b2[:], start=False, stop=True)
                    nc.tensor.matmul(out=Z1[:], lhsT=sqrow[:, 64:128],
                                     rhs=b2[:], start=False, stop=True)
                    h3 = agp.tile([128, H2], F32, tag="eh3")
                    nc.scalar.activation(out=h3[0:64, :], in_=Z0[:],
                                         func=mybir.ActivationFunctionType.Relu)
                    nc.scalar.activation(out=h3[64:128, :], in_=Z1[:],
                                         func=mybir.ActivationFunctionType.Relu)
                    g3sb = agp.tile([128, H2], F32R, tag="eg3sb")
                    nc.vector.tensor_scalar(
                        out=g3sb[:], in0=h3[:],
                        scalar1=deginvs[:, w : w + 1], scalar2=None,
                        op0=mybir.AluOpType.mult)
                    nc.sync.dma_start(
                        out=G3S[w * 128 : (w + 1) * 128, :].bitcast(F32R),
                        in_=g3sb[:])

            # ---------- phase F: AllGather table3 ----------
            if "F" in PHASES:
                nc.gpsimd.collective_compute(
                    "AllGather", mybir.AluOpType.bypass, replica_groups=rg,
                    ins=[G3S[:].opt()], outs=[G3F[:].opt()])

            # ---------- phase G: layer-3 aggregation + W3 + log_softmax ------
            with tc.tile_pool(name="ag3", bufs=2) as agp, \
                 tc.tile_pool(name="ag3z", bufs=2, space="PSUM") as agps, \
                 tc.tile_pool(name="ag3q", bufs=1, space="PSUM") as sqps, \
                 tc.tile_pool(name="ag3p", bufs=1, space="PSUM") as p3ps, \
                 tc.tile_pool(name="ag3t", bufs=2, space="PSUM") as trps:
                for w in range(NWC if "G" in PHASES else 0):
                    Z0, Z1 = aggregate(w, lambda a, b: G3F[a:b, :], H2, agp,
                                       agps, "g", stop_last=True)
                    z3 = agp.tile([128, H2], F32R, tag="gz3")
                    nc.scalar.activation(out=z3[0:64, :], in_=Z0[:],
                                         func=mybir.ActivationFunctionType.Copy)
                    nc.scalar.activation(out=z3[64:128, :], in_=Z1[:],
                                         func=mybir.ActivationFunctionType.Copy)
                    tp = trps.tile([H2, 128], F32R, space="PSUM", tag="gtp")
                    nc.tensor.transpose(out=tp[:], in_=z3[:], identity=ident[:])
                    z3T = agp.tile([H2, 128], F32R, tag="gz3T")
                    nc.vector.tensor_copy(out=z3T[:], in_=tp[:])
                    p3 = p3ps.tile([128, 16], F32, space="PSUM", tag="gp3")
                    nc.tensor.matmul(out=p3[:], lhsT=z3T[:], rhs=w3[:],
                                     start=True, stop=False)
                    sqrow = sqrtdeg_row(w, agp, sqps, "g")
                    nc.tensor.matmul(out=p3[:], lhsT=sqrow[:], rhs=b3[:],
                                     start=False, stop=True)
                    zf = agp.tile([128, 16], F32, tag="gzf")
                    nc.vector.tensor_scalar(
                        out=zf[:], in0=p3[:],
                        scalar1=dinvs[:, w : w + 1], scalar2=None,
                        op0=mybir.AluOpType.mult)
                    m = agp.tile([128, 1], F32, tag="gm")
                    nc.vector.reduce_max(out=m[:], in_=zf[:, :C],
                                         axis=mybir.AxisListType.X)
                    nc.vector.tensor_scalar(
                        out=TTA[:, w, :], in0=zf[:, :C], scalar1=m[:],
                        scalar2=None, op0=mybir.AluOpType.subtract)
                    ee = agp.tile([128, C], F32, tag="gee")
                    nc.scalar.activation(out=ee[:], in_=TTA[:, w, :],
                                         func=mybir.ActivationFunctionType.Exp,
                                         accum_out=SSA[:, w : w + 1])
                # single ln pass over all windows' exp-sums (one table load),
                # then subtract and store
                if "G" in PHASES:
                    nc.scalar.activation(out=LSA[:], in_=SSA[:],
                                         func=mybir.ActivationFunctionType.Ln)
                    nc.vector.tensor_tensor(
                        out=OOA[:],
                        in0=TTA[:],
                        in1=LSA[:].unsqueeze(2).to_broadcast([128, NWC, C]),
                        op=mybir.AluOpType.subtract)
                    for w in range(NWC):
                        nc.sync.dma_start(out=OUT[w * 128 : (w + 1) * 128, :],
                                          in_=OOA[:, w, :])

        if DEBUG_TAPS:
            DG1 = nc.dram_tensor("DG1", [cfg.NP, H1], F32, kind="ExternalOutput")
            DG2 = nc.dram_tensor("DG2", [SHARD, H2], F32, kind="ExternalOutput")
            DG3 = nc.dram_tensor("DG3", [SHARD, H2], F32, kind="ExternalOutput")
            nc.sync.dma_start(out=DG1[:], in_=G1[:])
            nc.sync.dma_start(out=DG2[:], in_=G2S[:])
            nc.sync.dma_start(out=DG3[:], in_=G3S[:])

    nc.compile()
    return nc


# --------------------------------------------------------------------------
# driver
# --------------------------------------------------------------------------
def _weights_layout(W1, W2, W3, b1, b2, b3, cfg):
    KB = cfg.F_IN // 128
    KB2 = cfg.H1 // 128
    W1t = np.ascontiguousarray(
        np.asarray(W1, np.float32).reshape(KB, 128, cfg.H1).transpose(1, 0, 2))
    W2t = np.ascontiguousarray(
        np.asarray(W2, np.float32).reshape(KB2, 128, cfg.H2).transpose(1, 0, 2))
    W3p = np.zeros((cfg.H2, 16), np.float32)
    W3p[:, : cfg.C] = np.asarray(W3, np.float32)
    B1 = np.asarray(b1, np.float32).reshape(1, cfg.H1)
    B2 = np.asarray(b2, np.float32).reshape(1, cfg.H2)
    B3 = np.zeros((1, 16), np.float32)
    B3[0, : cfg.C] = np.asarray(b3, np.float32)
    if USE_BF16:
        W1t = W1t.astype(mybir.dt.np(BF16))
    return W1t, W2t, W3p, B1, B2, B3


def prepare(x, edge_index, W1, b1, W2, b2, W3, b3, cfg=FULL_CFG):
    host, meta = _preprocess(x, edge_index, cfg)
    W1t, W2t, W3p, B1, B2, B3 = _weights_layout(W1, W2, W3, b1, b2, b3, cfg)
    nc = _build_program(cfg, meta)
    in_maps = []
    for c in range(N_CORES):
        in_maps.append({
            "XT": host["XT"], "W1": W1t, "W2": W2t, "W3": W3p,
            "B1": B1, "B2": B2, "B3": B3,
            "IOTA": host["iota"], "DEGF": host["deg_full"],
            "DEGS": host["deg_shard"][c],
            "IDX": host["idx_flat"][c], "IDX1": host["idx1_flat"][c],
            "DSTL": host["dstloc_flat"][c],
        })
    return nc, in_maps, host


def run(x, edge_index, W1, b1, W2, b2, W3, b3, cfg=FULL_CFG, trace=False):
    nc, in_maps, host = prepare(x, edge_index, W1, b1, W2, b2, W3, b3, cfg)
    res = bass_utils.run_bass_kernel_spmd(
        nc, in_maps, core_ids=list(range(N_CORES)), trace=trace)

    outp = np.concatenate([res.results[c]["OUT"] for c in range(N_CORES)], axis=0)
    out = outp[host["perm_row"][: cfg.N]]
    return out.astype(np.float32), res


def make_runner(cfg, nc, in_maps):
    """Build a reusable jitted 8-core runner with inputs resident on device.

    Returns (fn, args) where fn(*args) -> list of per-core OUT arrays; call
    repeatedly for timing without host->device transfer of the big inputs.
    """
    import jax
    from jax.sharding import Mesh, PartitionSpec
    from jax.experimental.shard_map import shard_map
    from concourse import bass2jax as b2j

    b2j.install_neuronx_cc_hook()
    partition_name = nc.partition_id_tensor.name if nc.partition_id_tensor else None
    in_names, out_names, out_avals, zero_outs = [], [], [], []
    for alloc in nc.m.functions[0].allocations:
        if not isinstance(alloc, mybir.MemoryLocationSet):
            continue
        name = alloc.memorylocations[0].name
        if alloc.kind == "ExternalInput":
            if name != partition_name:
                in_names.append(name)
        elif alloc.kind == "ExternalOutput":
            shape = tuple(alloc.tensor_shape)
            dtype = mybir.dt.np(alloc.dtype)
            out_names.append(name)
            out_avals.append(jax.core.ShapedArray(shape, dtype))
            zero_outs.append(np.zeros(shape, dtype))
    n_params = len(in_names)
    all_in_names = list(in_names) + out_names
    if partition_name is not None:
        all_in_names.append(partition_name)

    def _body(*args):
        operands = list(args)
        if partition_name is not None:
            operands.append(b2j.partition_id_tensor())
        outs = b2j._bass_exec_p.bind(
            *operands, out_avals=tuple(out_avals), in_names=tuple(all_in_names),
            out_names=tuple(out_names), lowering_input_output_aliases=(),
            sim_require_finite=False, sim_require_nnan=False, nc=nc)
        return tuple(outs)

    devices = jax.devices()[:N_CORES]
    mesh = Mesh(np.asarray(devices), ("core",))
    in_specs = (PartitionSpec("core"),) * (n_params + len(out_names))
    out_specs = (PartitionSpec("core"),) * len(out_names)
    sharded = jax.jit(
        shard_map(_body, mesh=mesh, in_specs=in_specs, out_specs=out_specs,
                  check_rep=False),
        keep_unused=True)
    sh = jax.sharding.NamedSharding(mesh, PartitionSpec("core"))
    concat_in = [
        jax.device_put(
            np.concatenate([np.asarray(in_maps[c][k]) for c in range(N_CORES)], 0),
            sh)
        for k in in_names
    ]
    concat_zeros = [
        jax.device_put(np.zeros((N_CORES * z.shape[0], *z.shape[1:]), z.dtype), sh)
        for z in zero_outs
    ]
    args = concat_in + concat_zeros

    def fn(*a):
        outs = sharded(*a)
        return {name: outs[i] for i, name in enumerate(out_names)}

    return fn, args, out_names


def kernel(**inputs):
    out, _ = run(
        inputs["x"], inputs["edge_index"],
        inputs["W1"], inputs["b1"], inputs["W2"], inputs["b2"],
        inputs["W3"], inputs["b3"], cfg=FULL_CFG)
    return out

